# revision 35
# baseline (speedup 1.0000x reference)
"""Cross-attention kernel for Trainium2, 8-core SPMD.

Problem (all fp32):
  x [2, 2048, 1024]; wq/wk/wv/w_proj [1024, 1024]; b_proj [1024]
  q = x[:, :1024] @ wq.T   (16 heads x 64)
  k, v = x @ wk.T, x @ wv.T
  out = softmax(q k^T / 8) v  -> proj + bias  -> [2, 1024, 1024]

Sharding: 8 cores = 2 (batch) x 4 (head-groups of 4 heads = 2 pairs of 2).
Each core emits TWO bf16 partials (one per head-pair); the host upcasts,
sums the 16 partials per batch and adds the bias (tensor-parallel unshard).

Design (cost model: matmul = out-cols x 0.4167ns x cpr; bf16 cpr=1 at any
width, fp32r cpr=4 below 256 cols; ACT ~1.04us per [128,1024] exp):
  - x/weights stream in as bf16 (half DMA bytes); q/k kept fp32r so scores
    stay high precision; exp output, v, attn, projection all bf16.
  - attnv is transposed: stationary = exp tile [128kv, 128q], moving =
    v [128kv, 65] -> psum [q-block, 65].  8320 cols/head vs 16384, and the
    ones-column denominator lands per-PARTITION, so normalization is a
    cheap DVE tensor_scalar multiply (no PE broadcast matmuls).
  - normalized attn for a head-pair is packed [128q, 128dd], PE-transposed
    and projected with a full-128 contraction (16384 cols vs 32768).
  - PSUM (8 banks): scores [128,1024] x2 (4) + attnv 2 x [128,4,65] (2) +
    two [128,512] half-slots (2) that 2-wide-pipeline every sequential
    filler (k1 quarters, v pairs, pair0 projection halves).  q pair1 runs
    as four [128,256] quarters inside the 1-bank attnv slots during stage
    A.  Tail projection runs at [128,256] quarter granularity across all
    three freed pools.
  - HW psum rule: start_tensor_calc lazily zeroes the whole 2KB bank, so
    only the FIRST group in a bank may use start=True; later groups in the
    same bank begin with start=False and rely on the pending-zero bytes
    (v pairs share a bank, attnv shares a bank across 4 q-blocks).
  - GPSIMD cannot touch PSUM: all psum evacuation is DVE/ACT; ACT is kept
    free for the 64-exp stream mid-kernel (it paces the whole kernel) and
    only joins evacuation pre-stream and in the tail.
  - The exp stream is levelled across the head loops with cost-aware
    filler pulls: h0 carries v j0..9 + q pair1 quarters, h1 carries k
    pair1 + v j10..13, h2 carries v j14..15 + attnv(h0) + attnv(h1) +
    pair0 transposes, h3 carries attnv(h2) + attnv(h3) + the pair0
    projection (streamed out mid-kernel through the half-slots).
"""

import os
import numpy as np
import ml_dtypes

import concourse.bacc as bacc
import concourse.bass as bass
import concourse.tile as tile
import concourse.mybir as mybir
from concourse.bass_utils import run_bass_kernel_spmd
from concourse.masks import make_identity

F32 = mybir.dt.float32
F32R = mybir.dt.float32r
BF16 = mybir.dt.bfloat16

C = 1024          # model dim
N = 2048          # kv tokens
NQ = 1024         # query tokens
HPC = 4           # heads per core
D = 64            # head dim
DH = HPC * D      # per-core slice of C (256)
SCALE = D ** -0.5
P = 128

_CACHE: dict = {}
_BF = ml_dtypes.bfloat16


def _build():
    nc = bacc.Bacc("TRN2", target_bir_lowering=False, debug=False, num_devices=8)

    xT = nc.dram_tensor("xT", [C, N], BF16, kind="ExternalInput").ap()
    # wqk{p} = hstack(wq[pair p slice].T, wk[pair p slice].T)  [C, 256]
    wqk0 = nc.dram_tensor("wqk0", [C, 2 * P], BF16, kind="ExternalInput").ap()
    wqk1 = nc.dram_tensor("wqk1", [C, 2 * P], BF16, kind="ExternalInput").ap()
    wvT = nc.dram_tensor("wvT", [C, DH], BF16, kind="ExternalInput").ap()
    wpT = nc.dram_tensor("wpT", [DH, C], BF16, kind="ExternalInput").ap()
    outA = nc.dram_tensor("outA", [NQ, C], BF16, kind="ExternalOutput").ap()
    outB = nc.dram_tensor("outB", [NQ, C], BF16, kind="ExternalOutput").ap()

    with tile.TileContext(nc) as tc, \
            nc.allow_low_precision(reason="bf16 pipeline within 2e-2 tolerance"):
        _emit(tc, xT, wqk0, wqk1, wvT, wpT, outA, outB)

    nc.compile()
    return nc


def _emit(tc, xT, wqk0, wqk1, wvT, wpT, outA, outB):
    nc = tc.nc
    mm = nc.tensor.matmul
    Exp = mybir.ActivationFunctionType.Exp
    Copy = mybir.ActivationFunctionType.Copy

    from contextlib import ExitStack
    from itertools import chain

    with ExitStack() as ctx:
        singles = ctx.enter_context(tc.tile_pool(name="singles", bufs=1))
        ets_pool = ctx.enter_context(tc.tile_pool(name="ets", bufs=32))
        finp = ctx.enter_context(tc.tile_pool(name="finp", bufs=8))
        ps_sc = ctx.enter_context(tc.tile_pool(name="ps_sc", bufs=2, space="PSUM"))
        ps_av = ctx.enter_context(tc.tile_pool(name="ps_av", bufs=2, space="PSUM"))
        ps_sq = ctx.enter_context(tc.tile_pool(name="ps_sq", bufs=2, space="PSUM"))

        # ---------------- input DMAs (one ordered SP/HWDGE stream) --------
        # Per chunk: wqk0_ci, wqk1_ci, x_ci  (stage A consumes q0/k0/q1 per
        # chunk as it lands); then wv, wp (needed from ~h0/h3).
        xt = [singles.tile([P, N], BF16, name=f"xt{ci}", tag=f"xt{ci}")
              for ci in range(8)]
        wqk0_sb = singles.tile([P, 8, 2 * P], BF16, name="wqk0_sb", tag="wqk0")
        wqk1_sb = singles.tile([P, 8, 2 * P], BF16, name="wqk1_sb", tag="wqk1")
        wv_sb = singles.tile([P, 8, DH], BF16, name="wv_sb", tag="wv")
        wp_sb = [singles.tile([P, C], BF16, name=f"wp{p}", tag=f"wp{p}")
                 for p in range(2)]

        wqk0_src = wqk0.rearrange("(a p) d -> p a d", p=P)
        wqk1_src = wqk1.rearrange("(a p) d -> p a d", p=P)
        wv_src = wvT.rearrange("(a p) d -> p a d", p=P)

        for ci in range(8):
            nc.sync.dma_start(out=wqk0_sb[:, ci, :], in_=wqk0_src[:, ci, :])
            if ci >= 6:
                nc.sync.dma_start(out=xt[ci][:, 0:1024],
                                  in_=xT[ci * P:(ci + 1) * P, 0:1024])
                nc.sync.dma_start(out=xt[ci][:, 1024:2048],
                                  in_=xT[ci * P:(ci + 1) * P, 1024:2048])
            else:
                nc.sync.dma_start(out=xt[ci], in_=xT[ci * P:(ci + 1) * P, :])
        for ci in range(8):
            nc.sync.dma_start(out=wqk1_sb[:, ci, :], in_=wqk1_src[:, ci, :])
            nc.sync.dma_start(out=wv_sb[:, ci, :], in_=wv_src[:, ci, :])
        for p in range(2):
            nc.sync.dma_start(out=wp_sb[p], in_=wpT[p * P:(p + 1) * P, :])

        # ---------------- small consts ----------------
        identity = singles.tile([P, P], BF16, name="identity", tag="ident")
        make_identity(nc, identity)

        # Pre-trigger the exp table load while DMAs stream.
        dmt = singles.tile([1, 1], BF16, name="dmt", tag="dmt")
        nc.scalar.activation(out=dmt, in_=identity[0:1, 0:1], func=Exp, scale=1.0)

        # ---------------- persistent SBUF ----------------
        qt = [singles.tile([P, NQ], F32R, name=f"qt{p}", tag=f"qt{p}")
              for p in range(2)]
        kt = [singles.tile([P, N], F32R, name=f"kt{p}", tag=f"kt{p}")
              for p in range(2)]
        v_sb = singles.tile([P, 16, HPC, D + 1], BF16, name="v_sb", tag="v_sb")
        nc.gpsimd.memset(v_sb[:, :, :, D:D + 1], 1.0)

        attn_pack = [singles.tile([P, 8, P], BF16, name=f"apk{p}", tag=f"apk{p}")
                     for p in range(2)]
        attn_T = [singles.tile([P, 8, P], BF16, name=f"atT{p}", tag=f"atT{p}")
                  for p in range(2)]
        rcp = singles.tile([P, HPC, 8], F32, name="rcp", tag="rcp")

        # ---------------- stage A: q pair0 + k pair0 + q1 quarters --------
        # 8 mm per chunk vs ~1.6us chunk arrival: roughly DMA-paced.  q1 is
        # split into four [128, 256] quarter-psums so two of them fit the
        # (otherwise idle) 1-bank attnv slots during stage A; the other two
        # run as the first h0 fillers.
        ps_q0a = ps_sq.tile([P, 512], F32, name="ps_q0a", tag="sq")
        ps_q0b = ps_sq.tile([P, 512], F32, name="ps_q0b", tag="sq")
        ps_k0a = ps_sc.tile([P, NQ], F32, name="ps_k0a", tag="sc")
        ps_k0b = ps_sc.tile([P, NQ], F32, name="ps_k0b", tag="sc")
        ps_q1a = ps_av.tile([P, 256], F32, name="ps_q1a", tag="av")
        ps_q1b = ps_av.tile([P, 256], F32, name="ps_q1b", tag="av")
        for ci in range(8):
            lw_q0 = wqk0_sb[:, ci, 0:P]
            lw_k0 = wqk0_sb[:, ci, P:2 * P]
            lw_q1 = wqk1_sb[:, ci, 0:P]
            st = dict(start=(ci == 0), stop=(ci == 7), skip_group_check=True)
            mm(ps_q0a, lw_q0, xt[ci][:, 0:512], **st)
            mm(ps_k0a[:, 0:512], lw_k0, xt[ci][:, 0:512], **st)
            mm(ps_q0b, lw_q0, xt[ci][:, 512:1024], **st)
            mm(ps_k0a[:, 512:1024], lw_k0, xt[ci][:, 512:1024], **st)
            mm(ps_k0b[:, 0:512], lw_k0, xt[ci][:, 1024:1536], **st)
            mm(ps_k0b[:, 512:1024], lw_k0, xt[ci][:, 1536:2048], **st)
            mm(ps_q1a, lw_q1, xt[ci][:, 0:256], **st)
            mm(ps_q1b, lw_q1, xt[ci][:, 256:512], **st)
        # evacs split across DVE+ACT+Pool (all idle pre-stream) to shorten
        # the serial path to the first scores matmul
        nc.vector.tensor_copy(qt[0][:, 0:512], ps_q0a)
        nc.scalar.copy(qt[0][:, 512:1024], ps_q0b)
        nc.vector.tensor_copy(kt[0][:, 0:512], ps_k0a[:, 0:512])
        nc.scalar.copy(kt[0][:, 512:1024], ps_k0a[:, 512:1024])
        nc.vector.tensor_copy(kt[0][:, 1024:1536], ps_k0b[:, 0:512])
        nc.scalar.copy(kt[0][:, 1536:2048], ps_k0b[:, 512:1024])
        nc.vector.tensor_copy(qt[1][:, 0:256], ps_q1a)
        nc.vector.tensor_copy(qt[1][:, 256:512], ps_q1b)

        # ---------------- fillers ----------------
        MM = 0.427  # us per 512-col matmul at full clock (cost bookkeeping)

        def q1cd_gen():
            # q1 quarters C/D through the freed attnv slots (xt resident)
            ps_c = ps_av.tile([P, 256], F32, name="ps_q1c", tag="av")
            for ci in range(8):
                mm(ps_c, wqk1_sb[:, ci, 0:P], xt[ci][:, 512:768],
                   start=(ci == 0), stop=(ci == 7), skip_group_check=True)
                yield 107
            ps_d = ps_av.tile([P, 256], F32, name="ps_q1d", tag="av")
            for ci in range(8):
                mm(ps_d, wqk1_sb[:, ci, 0:P], xt[ci][:, 768:1024],
                   start=(ci == 0), stop=(ci == 7), skip_group_check=True)
                yield 107
            nc.vector.tensor_copy(qt[1][:, 512:768], ps_c)
            nc.vector.tensor_copy(qt[1][:, 768:1024], ps_d)
            yield 0

        def k1_gen(quarter):
            ps = ps_sq.tile([P, 512], F32, name=f"ps_k1{quarter}", tag="sq")
            nk0 = quarter * 512
            for ci in range(8):
                lw = wqk1_sb[:, ci, P:2 * P]
                mm(ps, lw, xt[ci][:, nk0:nk0 + 512],
                   start=(ci == 0), stop=(ci == 7), skip_group_check=True)
                yield 213
            nc.vector.tensor_copy(kt[1][:, nk0:nk0 + 512], ps)
            yield 0

        def v_gen(t):
            # 2 kv-blocks (j = 2t, 2t+1) share one psum slot; one wide evac
            ps = ps_sq.tile([P, 2, DH], F32, name=f"ps_v{t}", tag="sq")
            for ci in range(8):
                for jj in range(2):
                    j = 2 * t + jj
                    # one start per psum BANK: jj=1's first matmul relies on
                    # the pending-zero left by jj=0's start
                    mm(ps[:, jj, :], xt[ci][:, j * P:(j + 1) * P],
                       wv_sb[:, ci, :],
                       start=(ci == 0 and jj == 0),
                       stop=(ci == 7 and jj == 1), skip_group_check=True)
                yield 214
            nc.vector.tensor_copy(
                v_sb[:, 2 * t:2 * t + 2, :, 0:D],
                ps.rearrange("p j (h d) -> p j h d", h=HPC))
            yield 0

        def proj0_gen(m):
            # two independent half-column units -> 2-wide through the sq slots
            for nh in range(2):
                ps = ps_sq.tile([P, 512], F32, name=f"ps_pj0_{m}_{nh}", tag="sq")
                mm(ps, attn_T[0][:, m, :],
                   wp_sb[0][:, nh * 512:(nh + 1) * 512],
                   start=True, stop=True, skip_group_check=True)
                yield 213
                fin = finp.tile([P, 512], BF16, name=f"fin0_{m}_{nh}", tag="fin")
                nc.vector.tensor_copy(fin, ps)
                nc.sync.dma_start(
                    out=outA[m * P:(m + 1) * P, nh * 512:(nh + 1) * 512],
                    in_=fin)
                yield 0

        # ---------------- attention pieces ----------------
        av_tiles = {}

        def alloc_av(h):
            av_tiles[h] = [ps_av.tile([P, 4, D + 1], F32, name=f"av{h}_{s}",
                                      tag="av") for s in range(2)]

        ets = {}

        def scores_j(h, j):
            pair, po = h // 2, D * (h % 2)
            ps = ps_sc.tile([P, NQ], F32, name=f"ps_s{h}_{j}", tag="sc")
            lw = kt[pair][po:po + D, j * P:(j + 1) * P]
            for nh in range(2):
                mm(ps[:, nh * 512:(nh + 1) * 512], lw,
                   qt[pair][po:po + D, nh * 512:(nh + 1) * 512],
                   start=True, stop=True, skip_group_check=True)
            et = ets_pool.tile([P, NQ], BF16, name=f"et{h}_{j}", tag="ets")
            nc.scalar.activation(out=et, in_=ps, func=Exp, scale=SCALE)
            ets[(h, j)] = et

        def attnv_j(h, j):
            et = ets[(h, j)]
            for qb in range(8):
                av = av_tiles[h][qb // 4]
                mm(av[:, qb % 4, :],
                   et[:, qb * P:(qb + 1) * P],
                   v_sb[:, j, h, :],
                   start=(j == 0 and qb % 4 == 0),
                   stop=(j == 15 and qb % 4 == 3),
                   skip_group_check=True)

        def norm_half(h, part, tail):
            # tail=False: DVE + Pool (ACT is mid-exp-stream); tail=True:
            # DVE + ACT (lower latency, stream over)
            pair, half = h // 2, h % 2
            av = av_tiles[h][part]
            nc.vector.reciprocal(rcp[:, h, part * 4:(part + 1) * 4], av[:, :, D])
            for i in range(4):
                qb = part * 4 + i
                dst = attn_pack[pair][:, qb, half * D:(half + 1) * D]
                if tail and i % 2 == 1:
                    nc.scalar.activation(out=dst, in_=av[:, i, 0:D], func=Copy,
                                         scale=rcp[:, h, qb:qb + 1])
                else:
                    nc.vector.tensor_scalar_mul(dst, av[:, i, 0:D],
                                                rcp[:, h, qb:qb + 1])

        def pull(gen, budget):
            # cost-aware: drain up to ~budget ns of emitted matmul work
            acc = 0
            while acc < budget:
                c = next(gen, None)
                if c is None:
                    return False
                acc += c
            return True

        # ---------------- head loops (ACT exp stream is the pacer) --------
        # Each head's 16 exps give ~17.1us of ACT; scores are ~6.8us of PE,
        # leaving ~640ns/iter of PE filler budget.
        # h0: v pairs 0..4 (j0..9) + q1 quarters C/D
        f = chain(*(v_gen(t) for t in range(5)), q1cd_gen())
        for j in range(16):
            scores_j(0, j)
            pull(f, 640)
        for _ in f:
            pass

        # h1: k1a + k1b first (unblocks h2 scores), then v pairs 5, 6
        f = chain(*(k1_gen(qu) for qu in range(4)), *(v_gen(t) for t in (5, 6)))
        for j in range(16):
            scores_j(1, j)
            pull(f, 600)
        for _ in f:
            pass

        # h2: v pair 7 early + attnv(h0) iters 0..4, norm(h0)@5,
        #     attnv(h1) 6..11, norm(h1)@12, pair0 transpose @12
        A0 = [(0, 3), (3, 6), (6, 9), (9, 11), (11, 14), (14, 16)]
        A1 = [(0, 3), (3, 5), (5, 8), (8, 10), (10, 13), (13, 16)]
        alloc_av(0)
        fv = chain(v_gen(7))
        fp = chain(*(proj0_gen(m) for m in range(8)))

        def tp0(qb):
            tp = ps_av.tile([P, P], BF16, name=f"tp0_{qb}", tag="av")
            nc.tensor.transpose(tp, attn_pack[0][:, qb, :], identity)
            nc.vector.tensor_copy(attn_T[0][:, qb, :], tp)

        for j in range(16):
            scores_j(2, j)
            if j < 6:
                for jj in range(*A0[j]):
                    attnv_j(0, jj)
            elif j == 6:
                norm_half(0, 0, False)
                norm_half(0, 1, False)
                alloc_av(1)
            elif j < 13:
                for jj in range(*A1[j - 7]):
                    attnv_j(1, jj)
            elif j == 13:
                norm_half(1, 0, False)
                norm_half(1, 1, False)
            elif j >= 14:
                for qb in (2 * (j - 14), 2 * (j - 14) + 1):
                    tp0(qb)
            if j < 4:
                pull(fv, 430)

        # h3: attnv(h2) iters 0..7, norm(h2)@8, attnv(h3) j0..14 iters 8..15,
        #     rest of proj0 spread over all iters
        alloc_av(2)
        for j in range(16):
            scores_j(3, j)
            if j < 2:
                tp0(4 + 2 * j)
                tp0(5 + 2 * j)
            if j < 8:
                attnv_j(2, 2 * j)
                attnv_j(2, 2 * j + 1)
            else:
                if j == 8:
                    norm_half(2, 0, False)
                    norm_half(2, 1, False)
                    alloc_av(3)
                for jj in range((j - 8) * 15 // 8, (j - 7) * 15 // 8):
                    attnv_j(3, jj)
            if 1 <= j <= 6:
                pull(fp, 520)
            elif j >= 10:
                pull(fp, 570)
        for _ in fp:
            pass

        # ---------------- tail ----------------
        attnv_j(3, 15)

        # Per-qb chain: normalize -> PE transpose -> evac -> project -> fin
        # -> DMA, with DVE/ACT/Pool round-robin so no single evac engine
        # serializes the drain.  proj psums 2-deep via the sc tag.
        av3 = av_tiles[3]
        nc.vector.reciprocal(rcp[:, 3, 0:4], av3[0][:, :, D])
        nc.vector.reciprocal(rcp[:, 3, 4:8], av3[1][:, :, D])

        def mul3(qb):
            dst = attn_pack[1][:, qb, D:2 * D]
            src_ = av3[qb // 4][:, qb % 4, 0:D]
            if qb % 2 == 0:
                nc.vector.tensor_scalar_mul(dst, src_, rcp[:, 3, qb:qb + 1])
            else:
                nc.scalar.activation(out=dst, in_=src_, func=Copy,
                                     scale=rcp[:, 3, qb:qb + 1])

        def tp1(qb):
            tp = ps_sq.tile([P, P], BF16, name=f"tp{qb}", tag="sq")
            nc.tensor.transpose(tp, attn_pack[1][:, qb, :], identity)
            if qb % 2 == 0:
                nc.vector.tensor_copy(attn_T[1][:, qb, :], tp)
            else:
                nc.scalar.copy(attn_T[1][:, qb, :], tp)

        # pair1 projection at quarter-column granularity: 32 independent
        # [128, 256] psum units spread across all three free pools (6 slots
        # in flight), single-engine fin per quarter (round-robin), one DMA
        # per m-block.
        qslots = [(ps_av, "av"), (ps_sc, "sc"), (ps_sq, "sq")]

        def proj1(m):
            fin = finp.tile([P, C], BF16, name=f"fin1_{m}", tag="fin")
            for qo in range(4):
                k = 4 * m + qo
                pool, tag = qslots[k % 3]
                ps = pool.tile([P, 256], F32, name=f"pj1_{m}_{qo}", tag=tag)
                mm(ps, attn_T[1][:, m, :],
                   wp_sb[1][:, qo * 256:(qo + 1) * 256],
                   start=True, stop=True, skip_group_check=True)
                dst = fin[:, qo * 256:(qo + 1) * 256]
                if k % 2 == 0:
                    nc.scalar.copy(dst, ps)
                else:
                    nc.vector.tensor_copy(dst, ps)
            nc.sync.dma_start(out=outB[m * P:(m + 1) * P, :], in_=fin)

        mul3(0)
        tp1(0)
        mul3(1)
        tp1(1)
        for qb in range(2, 8):
            mul3(qb)
            tp1(qb)
            proj1(qb - 2)
        proj1(6)
        proj1(7)


def _get_nc():
    if "nc" not in _CACHE:
        _CACHE["nc"] = _build()
    return _CACHE["nc"]


def kernel(x, wq, wk, wv, w_proj, b_proj):
    x = np.asarray(x, dtype=np.float32)
    wq = np.asarray(wq, dtype=np.float32)
    wk = np.asarray(wk, dtype=np.float32)
    wv = np.asarray(wv, dtype=np.float32)
    w_proj = np.asarray(w_proj, dtype=np.float32)
    b_proj = np.asarray(b_proj, dtype=np.float32)

    nc = _get_nc()
    in_maps = []
    for core in range(8):
        b, g = divmod(core, 4)
        s0 = g * DH
        p0 = slice(s0, s0 + P)            # pair0 rows (heads 4g, 4g+1)
        p1 = slice(s0 + P, s0 + 2 * P)    # pair1 rows
        sl = slice(s0, s0 + DH)
        in_maps.append({
            "xT": np.ascontiguousarray(x[b].T).astype(_BF),
            "wqk0": np.ascontiguousarray(
                np.hstack([wq[p0, :].T, wk[p0, :].T])).astype(_BF),
            "wqk1": np.ascontiguousarray(
                np.hstack([wq[p1, :].T, wk[p1, :].T])).astype(_BF),
            "wvT": np.ascontiguousarray(wv[sl, :].T).astype(_BF),
            "wpT": np.ascontiguousarray(w_proj[:, sl].T).astype(_BF),
        })

    res = run_bass_kernel_spmd(nc, in_maps, core_ids=list(range(8)),
                               trace=bool(int(os.environ.get("KERNEL_TRACE", "0"))))
    _CACHE["last_results"] = res
    acc = [np.zeros((NQ, C), np.float32) for _ in range(2)]
    for core in range(8):
        b = core // 4
        acc[b] += res.results[core]["outA"].astype(np.float32)
        acc[b] += res.results[core]["outB"].astype(np.float32)
    full = np.stack(acc)
    full += b_proj[None, None, :]
    return full.astype(np.float32)


# revision 39
# speedup vs baseline: 1.0007x; 1.0007x over previous
"""Cross-attention kernel for Trainium2, 8-core SPMD (v3: bf16 + transposed attnv).

Problem (all fp32):
  x [2, 2048, 1024]; wq/wk/wv/w_proj [1024, 1024]; b_proj [1024]
  q = x[:, :1024] @ wq.T   (16 heads x 64)
  k, v = x @ wk.T, x @ wv.T
  out = softmax(q k^T / 8) v  -> proj + bias  -> [2, 1024, 1024]

Sharding: 8 cores = 2 (batch) x 4 (head-groups of 4 heads = 2 pairs of 2).
Each core emits TWO bf16 partials (one per head-pair); host upcasts, sums
the 16 partials per batch and adds the bias.

Design (matmul cost = out-cols x 0.4167ns x cpr; bf16 cpr=1 at any width,
fp32r cpr=4 below 256 cols):
  - x/weights stream in as bf16 (half DMA bytes); q/k kept fp32r so scores
    stay high precision; exp output, v, attn, proj all bf16.
  - attnv is transposed: stationary = exp tile [128kv, 128q], moving =
    v [128kv, 65] -> psum [q-block, 65].  8320 cols/head vs 16384, and the
    ones-column denominator lands per-PARTITION, so normalization is a
    cheap DVE tensor_scalar multiply (no PE broadcast matmuls).
  - normalized attn for a head-pair is packed [128q, 128dd], transposed
    (pair0: DMA-xbar mid-kernel; pair1: PE transpose in the tail where
    PSUM is free) and projected with a full-128 contraction.
  - PSUM (8 banks): scores [128, 1024] x2 (4) + attnv 2 x [128, 4, 65]
    (2) + one [128, 1024] rotating "seq" slot (2) for k1a/k1b/v_j/proj0.
    q pair1 runs inside stage A (its own psum there is the seq slot's
    first user).  Tail projection alternates the sc and seq tags for
    2-deep pipelining.
  - The exp stream (64 x [128, 1024], ~1.07us each) is the pacing engine;
    PE in-loop work is levelled across the 4 head loops so ACT never
    starves: h0 carries v j0..11, h1 carries k1 + v j12..15 + attnv(h0),
    h2 carries attnv(h1), h3 carries attnv(h2) + attnv(h3) + proj0.
"""

import os
import numpy as np
import ml_dtypes

import concourse.bacc as bacc
import concourse.bass as bass
import concourse.tile as tile
import concourse.mybir as mybir
from concourse.bass_utils import run_bass_kernel_spmd
from concourse.masks import make_identity

F32 = mybir.dt.float32
F32R = mybir.dt.float32r
BF16 = mybir.dt.bfloat16

C = 1024          # model dim
N = 2048          # kv tokens
NQ = 1024         # query tokens
HPC = 4           # heads per core
D = 64            # head dim
DH = HPC * D      # per-core slice of C (256)
SCALE = D ** -0.5
P = 128

_CACHE: dict = {}
_BF = ml_dtypes.bfloat16


def _build():
    nc = bacc.Bacc("TRN2", target_bir_lowering=False, debug=False, num_devices=8)

    xT = nc.dram_tensor("xT", [C, N], BF16, kind="ExternalInput").ap()
    # wqk{p} = hstack(wq[pair p slice].T, wk[pair p slice].T)  [C, 256]
    wqk0 = nc.dram_tensor("wqk0", [C, 2 * P], BF16, kind="ExternalInput").ap()
    wqk1 = nc.dram_tensor("wqk1", [C, 2 * P], BF16, kind="ExternalInput").ap()
    wvT = nc.dram_tensor("wvT", [C, DH], BF16, kind="ExternalInput").ap()
    wpT = nc.dram_tensor("wpT", [DH, C], BF16, kind="ExternalInput").ap()
    outA = nc.dram_tensor("outA", [NQ, C], BF16, kind="ExternalOutput").ap()
    outB = nc.dram_tensor("outB", [NQ, C], BF16, kind="ExternalOutput").ap()

    with tile.TileContext(nc) as tc, \
            nc.allow_low_precision(reason="bf16 pipeline within 2e-2 tolerance"):
        _emit(tc, xT, wqk0, wqk1, wvT, wpT, outA, outB)

    nc.compile()
    return nc


def _emit(tc, xT, wqk0, wqk1, wvT, wpT, outA, outB):
    nc = tc.nc
    mm = nc.tensor.matmul
    Exp = mybir.ActivationFunctionType.Exp
    Copy = mybir.ActivationFunctionType.Copy

    from contextlib import ExitStack
    from itertools import chain

    with ExitStack() as ctx:
        singles = ctx.enter_context(tc.tile_pool(name="singles", bufs=1))
        ets_pool = ctx.enter_context(tc.tile_pool(name="ets", bufs=32))
        finp = ctx.enter_context(tc.tile_pool(name="finp", bufs=8))
        ps_sc = ctx.enter_context(tc.tile_pool(name="ps_sc", bufs=2, space="PSUM"))
        ps_av = ctx.enter_context(tc.tile_pool(name="ps_av", bufs=2, space="PSUM"))
        ps_sq = ctx.enter_context(tc.tile_pool(name="ps_sq", bufs=2, space="PSUM"))

        # ---------------- input DMAs (one ordered SP/HWDGE stream) --------
        # Per chunk: wqk0_ci, wqk1_ci, x_ci  (stage A consumes q0/k0/q1 per
        # chunk as it lands); then wv, wp (needed from ~h0/h3).
        xt = [singles.tile([P, N], BF16, name=f"xt{ci}", tag=f"xt{ci}")
              for ci in range(8)]
        wqk0_sb = singles.tile([P, 8, 2 * P], BF16, name="wqk0_sb", tag="wqk0")
        wqk1_sb = singles.tile([P, 8, 2 * P], BF16, name="wqk1_sb", tag="wqk1")
        wv_sb = singles.tile([P, 8, DH], BF16, name="wv_sb", tag="wv")
        wp_sb = [singles.tile([P, C], BF16, name=f"wp{p}", tag=f"wp{p}")
                 for p in range(2)]

        wqk0_src = wqk0.rearrange("(a p) d -> p a d", p=P)
        wqk1_src = wqk1.rearrange("(a p) d -> p a d", p=P)
        wv_src = wvT.rearrange("(a p) d -> p a d", p=P)

        for ci in range(8):
            nc.sync.dma_start(out=wqk0_sb[:, ci, :], in_=wqk0_src[:, ci, :])
            if ci >= 6:
                nc.sync.dma_start(out=xt[ci][:, 0:1024],
                                  in_=xT[ci * P:(ci + 1) * P, 0:1024])
                nc.sync.dma_start(out=xt[ci][:, 1024:2048],
                                  in_=xT[ci * P:(ci + 1) * P, 1024:2048])
            else:
                nc.sync.dma_start(out=xt[ci], in_=xT[ci * P:(ci + 1) * P, :])
        for ci in range(8):
            nc.sync.dma_start(out=wqk1_sb[:, ci, :], in_=wqk1_src[:, ci, :])
            nc.sync.dma_start(out=wv_sb[:, ci, :], in_=wv_src[:, ci, :])
        for p in range(2):
            nc.sync.dma_start(out=wp_sb[p], in_=wpT[p * P:(p + 1) * P, :])

        # ---------------- small consts ----------------
        identity = singles.tile([P, P], BF16, name="identity", tag="ident")
        make_identity(nc, identity)

        # Pre-trigger the exp table load while DMAs stream.
        dmt = singles.tile([1, 1], BF16, name="dmt", tag="dmt")
        nc.scalar.activation(out=dmt, in_=identity[0:1, 0:1], func=Exp, scale=1.0)

        # ---------------- persistent SBUF ----------------
        qt = [singles.tile([P, NQ], F32R, name=f"qt{p}", tag=f"qt{p}")
              for p in range(2)]
        kt = [singles.tile([P, N], F32R, name=f"kt{p}", tag=f"kt{p}")
              for p in range(2)]
        v_sb = singles.tile([P, 16, HPC, D + 1], BF16, name="v_sb", tag="v_sb")
        nc.gpsimd.memset(v_sb[:, :, :, D:D + 1], 1.0)

        attn_pack = [singles.tile([P, 8, P], BF16, name=f"apk{p}", tag=f"apk{p}")
                     for p in range(2)]
        attn_T = [singles.tile([P, 8, P], BF16, name=f"atT{p}", tag=f"atT{p}")
                  for p in range(2)]
        rcp = singles.tile([P, HPC, 8], F32, name="rcp", tag="rcp")

        # ---------------- stage A: q pair0 + k pair0 + q1 quarters --------
        # 8 mm per chunk vs ~1.6us chunk arrival: roughly DMA-paced.  q1 is
        # split into four [128, 256] quarter-psums so two of them fit the
        # (otherwise idle) 1-bank attnv slots during stage A; the other two
        # run as the first h0 fillers.
        ps_q0a = ps_sq.tile([P, 512], F32, name="ps_q0a", tag="sq")
        ps_q0b = ps_sq.tile([P, 512], F32, name="ps_q0b", tag="sq")
        ps_k0a = ps_sc.tile([P, NQ], F32, name="ps_k0a", tag="sc")
        ps_k0b = ps_sc.tile([P, NQ], F32, name="ps_k0b", tag="sc")
        ps_q1a = ps_av.tile([P, 256], F32, name="ps_q1a", tag="av")
        ps_q1b = ps_av.tile([P, 256], F32, name="ps_q1b", tag="av")
        for ci in range(8):
            lw_q0 = wqk0_sb[:, ci, 0:P]
            lw_k0 = wqk0_sb[:, ci, P:2 * P]
            lw_q1 = wqk1_sb[:, ci, 0:P]
            st = dict(start=(ci == 0), stop=(ci == 7), skip_group_check=True)
            mm(ps_q0a, lw_q0, xt[ci][:, 0:512], **st)
            mm(ps_k0a[:, 0:512], lw_k0, xt[ci][:, 0:512], **st)
            mm(ps_q0b, lw_q0, xt[ci][:, 512:1024], **st)
            mm(ps_k0a[:, 512:1024], lw_k0, xt[ci][:, 512:1024], **st)
            mm(ps_k0b[:, 0:512], lw_k0, xt[ci][:, 1024:1536], **st)
            mm(ps_k0b[:, 512:1024], lw_k0, xt[ci][:, 1536:2048], **st)
            mm(ps_q1a, lw_q1, xt[ci][:, 0:256], **st)
            mm(ps_q1b, lw_q1, xt[ci][:, 256:512], **st)
        # evacs split across DVE+ACT+Pool (all idle pre-stream) to shorten
        # the serial path to the first scores matmul
        nc.vector.tensor_copy(qt[0][:, 0:512], ps_q0a)
        nc.scalar.copy(qt[0][:, 512:1024], ps_q0b)
        nc.vector.tensor_copy(kt[0][:, 0:512], ps_k0a[:, 0:512])
        nc.scalar.copy(kt[0][:, 512:1024], ps_k0a[:, 512:1024])
        nc.vector.tensor_copy(kt[0][:, 1024:1536], ps_k0b[:, 0:512])
        nc.scalar.copy(kt[0][:, 1536:2048], ps_k0b[:, 512:1024])
        nc.vector.tensor_copy(qt[1][:, 0:256], ps_q1a)
        nc.vector.tensor_copy(qt[1][:, 256:512], ps_q1b)

        # ---------------- fillers ----------------
        MM = 0.427  # us per 512-col matmul at full clock (cost bookkeeping)

        def q1cd_gen():
            # q1 quarters C/D through the freed attnv slots (xt resident)
            ps_c = ps_av.tile([P, 256], F32, name="ps_q1c", tag="av")
            for ci in range(8):
                mm(ps_c, wqk1_sb[:, ci, 0:P], xt[ci][:, 512:768],
                   start=(ci == 0), stop=(ci == 7), skip_group_check=True)
                yield 107
            ps_d = ps_av.tile([P, 256], F32, name="ps_q1d", tag="av")
            for ci in range(8):
                mm(ps_d, wqk1_sb[:, ci, 0:P], xt[ci][:, 768:1024],
                   start=(ci == 0), stop=(ci == 7), skip_group_check=True)
                yield 107
            nc.vector.tensor_copy(qt[1][:, 512:768], ps_c)
            nc.vector.tensor_copy(qt[1][:, 768:1024], ps_d)
            yield 0

        def k1_gen(quarter):
            ps = ps_sq.tile([P, 512], F32, name=f"ps_k1{quarter}", tag="sq")
            nk0 = quarter * 512
            for ci in range(8):
                lw = wqk1_sb[:, ci, P:2 * P]
                mm(ps, lw, xt[ci][:, nk0:nk0 + 512],
                   start=(ci == 0), stop=(ci == 7), skip_group_check=True)
                yield 213
            nc.vector.tensor_copy(kt[1][:, nk0:nk0 + 512], ps)
            yield 0

        def v_gen(t):
            # 2 kv-blocks (j = 2t, 2t+1) share one psum slot; one wide evac
            ps = ps_sq.tile([P, 2, DH], F32, name=f"ps_v{t}", tag="sq")
            for ci in range(8):
                for jj in range(2):
                    j = 2 * t + jj
                    # one start per psum BANK: jj=1's first matmul relies on
                    # the pending-zero left by jj=0's start
                    mm(ps[:, jj, :], xt[ci][:, j * P:(j + 1) * P],
                       wv_sb[:, ci, :],
                       start=(ci == 0 and jj == 0),
                       stop=(ci == 7 and jj == 1), skip_group_check=True)
                yield 214
            nc.vector.tensor_copy(
                v_sb[:, 2 * t:2 * t + 2, :, 0:D],
                ps.rearrange("p j (h d) -> p j h d", h=HPC))
            yield 0

        def proj0_gen(m):
            # two independent half-column units -> 2-wide through the sq slots
            for nh in range(2):
                ps = ps_sq.tile([P, 512], F32, name=f"ps_pj0_{m}_{nh}", tag="sq")
                mm(ps, attn_T[0][:, m, :],
                   wp_sb[0][:, nh * 512:(nh + 1) * 512],
                   start=True, stop=True, skip_group_check=True)
                yield 213
                fin = finp.tile([P, 512], BF16, name=f"fin0_{m}_{nh}", tag="fin")
                nc.vector.tensor_copy(fin, ps)
                nc.sync.dma_start(
                    out=outA[m * P:(m + 1) * P, nh * 512:(nh + 1) * 512],
                    in_=fin)
                yield 0

        # ---------------- attention pieces ----------------
        av_tiles = {}

        def alloc_av(h):
            av_tiles[h] = [ps_av.tile([P, 4, D + 1], F32, name=f"av{h}_{s}",
                                      tag="av") for s in range(2)]

        ets = {}

        def scores_j(h, j):
            pair, po = h // 2, D * (h % 2)
            ps = ps_sc.tile([P, NQ], F32, name=f"ps_s{h}_{j}", tag="sc")
            lw = kt[pair][po:po + D, j * P:(j + 1) * P]
            for nh in range(2):
                mm(ps[:, nh * 512:(nh + 1) * 512], lw,
                   qt[pair][po:po + D, nh * 512:(nh + 1) * 512],
                   start=True, stop=True, skip_group_check=True)
            et = ets_pool.tile([P, NQ], BF16, name=f"et{h}_{j}", tag="ets")
            nc.scalar.activation(out=et, in_=ps, func=Exp, scale=SCALE)
            ets[(h, j)] = et

        def attnv_j(h, j):
            et = ets[(h, j)]
            for qb in range(8):
                av = av_tiles[h][qb // 4]
                mm(av[:, qb % 4, :],
                   et[:, qb * P:(qb + 1) * P],
                   v_sb[:, j, h, :],
                   start=(j == 0 and qb % 4 == 0),
                   stop=(j == 15 and qb % 4 == 3),
                   skip_group_check=True)

        def norm_half(h, part, tail):
            # tail=False: DVE + Pool (ACT is mid-exp-stream); tail=True:
            # DVE + ACT (lower latency, stream over)
            pair, half = h // 2, h % 2
            av = av_tiles[h][part]
            nc.vector.reciprocal(rcp[:, h, part * 4:(part + 1) * 4], av[:, :, D])
            for i in range(4):
                qb = part * 4 + i
                dst = attn_pack[pair][:, qb, half * D:(half + 1) * D]
                if tail and i % 2 == 1:
                    nc.scalar.activation(out=dst, in_=av[:, i, 0:D], func=Copy,
                                         scale=rcp[:, h, qb:qb + 1])
                else:
                    nc.vector.tensor_scalar_mul(dst, av[:, i, 0:D],
                                                rcp[:, h, qb:qb + 1])

        def pull(gen, budget):
            # cost-aware: drain up to ~budget ns of emitted matmul work
            acc = 0
            while acc < budget:
                c = next(gen, None)
                if c is None:
                    return False
                acc += c
            return True

        # ---------------- head loops (ACT exp stream is the pacer) --------
        # Each head's 16 exps give ~17.1us of ACT; scores are ~6.8us of PE,
        # leaving ~640ns/iter of PE filler budget.
        # h0: v pairs 0..4 (j0..9) + q1 quarters C/D
        f = chain(*(v_gen(t) for t in range(5)), q1cd_gen())
        for j in range(16):
            scores_j(0, j)
            pull(f, 640)
        for _ in f:
            pass

        # h1: k1a + k1b first (unblocks h2 scores), then v pairs 5, 6
        f = chain(*(k1_gen(qu) for qu in range(4)), *(v_gen(t) for t in (5, 6)))
        for j in range(16):
            scores_j(1, j)
            pull(f, 615)
        for _ in f:
            pass

        # h2: v pair 7 early + attnv(h0) iters 0..4, norm(h0)@5,
        #     attnv(h1) 6..11, norm(h1)@12, pair0 transpose @12
        A0 = [(0, 1), (1, 4), (4, 7), (7, 10), (10, 13), (13, 16)]
        A1 = [(0, 3), (3, 5), (5, 8), (8, 10), (10, 13), (13, 16)]
        alloc_av(0)
        fv = chain(v_gen(7))
        fp = chain(*(proj0_gen(m) for m in range(8)))

        def tp0(qb):
            tp = ps_av.tile([P, P], BF16, name=f"tp0_{qb}", tag="av")
            nc.tensor.transpose(tp, attn_pack[0][:, qb, :], identity)
            nc.vector.tensor_copy(attn_T[0][:, qb, :], tp)

        for j in range(16):
            scores_j(2, j)
            if j < 6:
                for jj in range(*A0[j]):
                    attnv_j(0, jj)
            elif j == 6:
                norm_half(0, 0, False)
                norm_half(0, 1, False)
                alloc_av(1)
            elif j < 13:
                for jj in range(*A1[j - 7]):
                    attnv_j(1, jj)
            elif j == 13:
                norm_half(1, 0, False)
                norm_half(1, 1, False)
            elif j >= 14:
                for qb in (2 * (j - 14), 2 * (j - 14) + 1):
                    tp0(qb)
            if j < 4:
                pull(fv, 430)

        # h3: attnv(h2) iters 0..7, norm(h2)@8, attnv(h3) j0..14 iters 8..15,
        #     rest of proj0 spread over all iters
        alloc_av(2)
        for j in range(16):
            scores_j(3, j)
            if j < 2:
                tp0(4 + 2 * j)
                tp0(5 + 2 * j)
            if j < 8:
                attnv_j(2, 2 * j)
                attnv_j(2, 2 * j + 1)
            else:
                if j == 8:
                    norm_half(2, 0, False)
                    norm_half(2, 1, False)
                    alloc_av(3)
                for jj in range((j - 8) * 15 // 8, (j - 7) * 15 // 8):
                    attnv_j(3, jj)
            if 1 <= j <= 6:
                pull(fp, 520)
            elif j >= 10:
                pull(fp, 570)
        for _ in fp:
            pass

        # ---------------- tail ----------------
        attnv_j(3, 15)

        # Per-qb chain: normalize -> PE transpose -> evac -> project -> fin
        # -> DMA, with DVE/ACT/Pool round-robin so no single evac engine
        # serializes the drain.  proj psums 2-deep via the sc tag.
        av3 = av_tiles[3]
        nc.vector.reciprocal(rcp[:, 3, 0:4], av3[0][:, :, D])
        nc.vector.reciprocal(rcp[:, 3, 4:8], av3[1][:, :, D])

        def mul3(qb):
            dst = attn_pack[1][:, qb, D:2 * D]
            src_ = av3[qb // 4][:, qb % 4, 0:D]
            if qb % 2 == 0:
                nc.vector.tensor_scalar_mul(dst, src_, rcp[:, 3, qb:qb + 1])
            else:
                nc.scalar.activation(out=dst, in_=src_, func=Copy,
                                     scale=rcp[:, 3, qb:qb + 1])

        def tp1(qb):
            tp = ps_sq.tile([P, P], BF16, name=f"tp{qb}", tag="sq")
            nc.tensor.transpose(tp, attn_pack[1][:, qb, :], identity)
            if qb % 2 == 0:
                nc.vector.tensor_copy(attn_T[1][:, qb, :], tp)
            else:
                nc.scalar.copy(attn_T[1][:, qb, :], tp)

        # pair1 projection at quarter-column granularity: 32 independent
        # [128, 256] psum units spread across all three free pools (6 slots
        # in flight), single-engine fin per quarter (round-robin), one DMA
        # per m-block.
        qslots = [(ps_av, "av"), (ps_sc, "sc"), (ps_sq, "sq")]

        def proj1(m):
            fin = finp.tile([P, C], BF16, name=f"fin1_{m}", tag="fin")
            for qo in range(4):
                k = 4 * m + qo
                pool, tag = qslots[k % 3]
                ps = pool.tile([P, 256], F32, name=f"pj1_{m}_{qo}", tag=tag)
                mm(ps, attn_T[1][:, m, :],
                   wp_sb[1][:, qo * 256:(qo + 1) * 256],
                   start=True, stop=True, skip_group_check=True)
                dst = fin[:, qo * 256:(qo + 1) * 256]
                if k % 2 == 0:
                    nc.scalar.copy(dst, ps)
                else:
                    nc.vector.tensor_copy(dst, ps)
            nc.sync.dma_start(out=outB[m * P:(m + 1) * P, :], in_=fin)

        mul3(0)
        tp1(0)
        mul3(1)
        tp1(1)
        for qb in range(2, 8):
            mul3(qb)
            tp1(qb)
            proj1(qb - 2)
        proj1(6)
        proj1(7)


def _get_nc():
    if "nc" not in _CACHE:
        _CACHE["nc"] = _build()
    return _CACHE["nc"]


def kernel(x, wq, wk, wv, w_proj, b_proj):
    x = np.asarray(x, dtype=np.float32)
    wq = np.asarray(wq, dtype=np.float32)
    wk = np.asarray(wk, dtype=np.float32)
    wv = np.asarray(wv, dtype=np.float32)
    w_proj = np.asarray(w_proj, dtype=np.float32)
    b_proj = np.asarray(b_proj, dtype=np.float32)

    nc = _get_nc()
    in_maps = []
    for core in range(8):
        b, g = divmod(core, 4)
        s0 = g * DH
        p0 = slice(s0, s0 + P)            # pair0 rows (heads 4g, 4g+1)
        p1 = slice(s0 + P, s0 + 2 * P)    # pair1 rows
        sl = slice(s0, s0 + DH)
        in_maps.append({
            "xT": np.ascontiguousarray(x[b].T).astype(_BF),
            "wqk0": np.ascontiguousarray(
                np.hstack([wq[p0, :].T, wk[p0, :].T])).astype(_BF),
            "wqk1": np.ascontiguousarray(
                np.hstack([wq[p1, :].T, wk[p1, :].T])).astype(_BF),
            "wvT": np.ascontiguousarray(wv[sl, :].T).astype(_BF),
            "wpT": np.ascontiguousarray(w_proj[:, sl].T).astype(_BF),
        })

    res = run_bass_kernel_spmd(nc, in_maps, core_ids=list(range(8)),
                               trace=bool(int(os.environ.get("KERNEL_TRACE", "0"))))
    _CACHE["last_results"] = res
    acc = [np.zeros((NQ, C), np.float32) for _ in range(2)]
    for core in range(8):
        b = core // 4
        acc[b] += res.results[core]["outA"].astype(np.float32)
        acc[b] += res.results[core]["outB"].astype(np.float32)
    full = np.stack(acc)
    full += b_proj[None, None, :]
    return full.astype(np.float32)


# revision 46
# speedup vs baseline: 1.0051x; 1.0044x over previous
"""Cross-attention kernel for Trainium2, 8-core SPMD (v3: bf16 + transposed attnv).

Problem (all fp32):
  x [2, 2048, 1024]; wq/wk/wv/w_proj [1024, 1024]; b_proj [1024]
  q = x[:, :1024] @ wq.T   (16 heads x 64)
  k, v = x @ wk.T, x @ wv.T
  out = softmax(q k^T / 8) v  -> proj + bias  -> [2, 1024, 1024]

Sharding: 8 cores = 2 (batch) x 4 (head-groups of 4 heads = 2 pairs of 2).
Each core emits TWO bf16 partials (one per head-pair); host upcasts, sums
the 16 partials per batch and adds the bias.

Design (matmul cost = out-cols x 0.4167ns x cpr; bf16 cpr=1 at any width,
fp32r cpr=4 below 256 cols):
  - x/weights stream in as bf16 (half DMA bytes); q/k kept fp32r so scores
    stay high precision; exp output, v, attn, proj all bf16.
  - attnv is transposed: stationary = exp tile [128kv, 128q], moving =
    v [128kv, 65] -> psum [q-block, 65].  8320 cols/head vs 16384, and the
    ones-column denominator lands per-PARTITION, so normalization is a
    cheap DVE tensor_scalar multiply (no PE broadcast matmuls).
  - normalized attn for a head-pair is packed [128q, 128dd], transposed
    (pair0: DMA-xbar mid-kernel; pair1: PE transpose in the tail where
    PSUM is free) and projected with a full-128 contraction.
  - PSUM (8 banks): scores [128, 1024] x2 (4) + attnv 2 x [128, 4, 65]
    (2) + one [128, 1024] rotating "seq" slot (2) for k1a/k1b/v_j/proj0.
    q pair1 runs inside stage A (its own psum there is the seq slot's
    first user).  Tail projection alternates the sc and seq tags for
    2-deep pipelining.
  - The exp stream (64 x [128, 1024], ~1.07us each) is the pacing engine;
    PE in-loop work is levelled across the 4 head loops so ACT never
    starves: h0 carries v j0..11, h1 carries k1 + v j12..15 + attnv(h0),
    h2 carries attnv(h1), h3 carries attnv(h2) + attnv(h3) + proj0.
"""

import os
import numpy as np
import ml_dtypes

import concourse.bacc as bacc
import concourse.bass as bass
import concourse.tile as tile
import concourse.mybir as mybir
from concourse.bass_utils import run_bass_kernel_spmd
from concourse.masks import make_identity

F32 = mybir.dt.float32
F32R = mybir.dt.float32r
BF16 = mybir.dt.bfloat16

C = 1024          # model dim
N = 2048          # kv tokens
NQ = 1024         # query tokens
HPC = 4           # heads per core
D = 64            # head dim
DH = HPC * D      # per-core slice of C (256)
SCALE = D ** -0.5
P = 128

_CACHE: dict = {}
_BF = ml_dtypes.bfloat16


def _build():
    nc = bacc.Bacc("TRN2", target_bir_lowering=False, debug=False, num_devices=8)

    xT = nc.dram_tensor("xT", [C, N], BF16, kind="ExternalInput").ap()
    # wqk{p} = hstack(wq[pair p slice].T, wk[pair p slice].T)  [C, 256]
    wqk0 = nc.dram_tensor("wqk0", [C, 2 * P], BF16, kind="ExternalInput").ap()
    wqk1 = nc.dram_tensor("wqk1", [C, 2 * P], BF16, kind="ExternalInput").ap()
    wvT = nc.dram_tensor("wvT", [C, DH], BF16, kind="ExternalInput").ap()
    wpT = nc.dram_tensor("wpT", [DH, C], BF16, kind="ExternalInput").ap()
    outA = nc.dram_tensor("outA", [NQ, C], BF16, kind="ExternalOutput").ap()
    outB = nc.dram_tensor("outB", [NQ, C], BF16, kind="ExternalOutput").ap()

    with tile.TileContext(nc) as tc, \
            nc.allow_low_precision(reason="bf16 pipeline within 2e-2 tolerance"):
        _emit(tc, xT, wqk0, wqk1, wvT, wpT, outA, outB)

    nc.compile()
    return nc


def _emit(tc, xT, wqk0, wqk1, wvT, wpT, outA, outB):
    nc = tc.nc
    mm = nc.tensor.matmul
    Exp = mybir.ActivationFunctionType.Exp
    Copy = mybir.ActivationFunctionType.Copy

    from contextlib import ExitStack
    from itertools import chain

    with ExitStack() as ctx:
        singles = ctx.enter_context(tc.tile_pool(name="singles", bufs=1))
        ets_pool = ctx.enter_context(tc.tile_pool(name="ets", bufs=32))
        finp = ctx.enter_context(tc.tile_pool(name="finp", bufs=8))
        ps_sc = ctx.enter_context(tc.tile_pool(name="ps_sc", bufs=2, space="PSUM"))
        ps_av = ctx.enter_context(tc.tile_pool(name="ps_av", bufs=2, space="PSUM"))
        ps_sq = ctx.enter_context(tc.tile_pool(name="ps_sq", bufs=2, space="PSUM"))

        # ---------------- input DMAs (one ordered SP/HWDGE stream) --------
        # Per chunk: wqk0_ci, wqk1_ci, x_ci  (stage A consumes q0/k0/q1 per
        # chunk as it lands); then wv, wp (needed from ~h0/h3).
        xt = [singles.tile([P, N], BF16, name=f"xt{ci}", tag=f"xt{ci}")
              for ci in range(8)]
        wqk0_sb = singles.tile([P, 8, 2 * P], BF16, name="wqk0_sb", tag="wqk0")
        wqk1_sb = singles.tile([P, 8, 2 * P], BF16, name="wqk1_sb", tag="wqk1")
        wv_sb = singles.tile([P, 8, DH], BF16, name="wv_sb", tag="wv")
        wp_sb = [singles.tile([P, C], BF16, name=f"wp{p}", tag=f"wp{p}")
                 for p in range(2)]

        wqk0_src = wqk0.rearrange("(a p) d -> p a d", p=P)
        wqk1_src = wqk1.rearrange("(a p) d -> p a d", p=P)
        wv_src = wvT.rearrange("(a p) d -> p a d", p=P)

        for ci in range(8):
            nc.sync.dma_start(out=wqk0_sb[:, ci, :], in_=wqk0_src[:, ci, :])
            if ci >= 6:
                nc.sync.dma_start(out=xt[ci][:, 0:1024],
                                  in_=xT[ci * P:(ci + 1) * P, 0:1024])
                nc.sync.dma_start(out=xt[ci][:, 1024:2048],
                                  in_=xT[ci * P:(ci + 1) * P, 1024:2048])
            else:
                nc.sync.dma_start(out=xt[ci], in_=xT[ci * P:(ci + 1) * P, :])
        for ci in range(8):
            nc.sync.dma_start(out=wqk1_sb[:, ci, :], in_=wqk1_src[:, ci, :])
            nc.sync.dma_start(out=wv_sb[:, ci, :], in_=wv_src[:, ci, :])
        for p in range(2):
            nc.sync.dma_start(out=wp_sb[p], in_=wpT[p * P:(p + 1) * P, :])

        # ---------------- small consts ----------------
        identity = singles.tile([P, P], BF16, name="identity", tag="ident")
        make_identity(nc, identity)

        # Pre-trigger the exp table load while DMAs stream.
        dmt = singles.tile([1, 1], BF16, name="dmt", tag="dmt")
        nc.scalar.activation(out=dmt, in_=identity[0:1, 0:1], func=Exp, scale=1.0)

        # ---------------- persistent SBUF ----------------
        qt = [singles.tile([P, NQ], F32R, name=f"qt{p}", tag=f"qt{p}")
              for p in range(2)]
        kt = [singles.tile([P, N], F32R, name=f"kt{p}", tag=f"kt{p}")
              for p in range(2)]
        v_sb = singles.tile([P, 16, HPC, D + 1], BF16, name="v_sb", tag="v_sb")
        nc.gpsimd.memset(v_sb[:, :, :, D:D + 1], 1.0)

        attn_pack = [singles.tile([P, 8, P], BF16, name=f"apk{p}", tag=f"apk{p}")
                     for p in range(2)]
        attn_T = [singles.tile([P, 8, P], BF16, name=f"atT{p}", tag=f"atT{p}")
                  for p in range(2)]
        rcp = singles.tile([P, HPC, 8], F32, name="rcp", tag="rcp")

        # ---------------- stage A: q pair0 + k pair0 + q1 quarters --------
        # 8 mm per chunk vs ~1.6us chunk arrival: roughly DMA-paced.  q1 is
        # split into four [128, 256] quarter-psums so two of them fit the
        # (otherwise idle) 1-bank attnv slots during stage A; the other two
        # run as the first h0 fillers.
        ps_q0a = ps_sq.tile([P, 512], F32, name="ps_q0a", tag="sq")
        ps_q0b = ps_sq.tile([P, 512], F32, name="ps_q0b", tag="sq")
        ps_k0a = ps_sc.tile([P, NQ], F32, name="ps_k0a", tag="sc")
        ps_k0b = ps_sc.tile([P, NQ], F32, name="ps_k0b", tag="sc")
        ps_q1a = ps_av.tile([P, 256], F32, name="ps_q1a", tag="av")
        ps_q1b = ps_av.tile([P, 256], F32, name="ps_q1b", tag="av")
        for ci in range(8):
            lw_q0 = wqk0_sb[:, ci, 0:P]
            lw_k0 = wqk0_sb[:, ci, P:2 * P]
            lw_q1 = wqk1_sb[:, ci, 0:P]
            st = dict(start=(ci == 0), stop=(ci == 7), skip_group_check=True)
            mm(ps_q0a, lw_q0, xt[ci][:, 0:512], **st)
            mm(ps_k0a[:, 0:512], lw_k0, xt[ci][:, 0:512], **st)
            mm(ps_q0b, lw_q0, xt[ci][:, 512:1024], **st)
            mm(ps_k0a[:, 512:1024], lw_k0, xt[ci][:, 512:1024], **st)
            mm(ps_k0b[:, 0:512], lw_k0, xt[ci][:, 1024:1536], **st)
            mm(ps_k0b[:, 512:1024], lw_k0, xt[ci][:, 1536:2048], **st)
            mm(ps_q1a, lw_q1, xt[ci][:, 0:256], **st)
            mm(ps_q1b, lw_q1, xt[ci][:, 256:512], **st)
        # evacs split across DVE+ACT+Pool (all idle pre-stream) to shorten
        # the serial path to the first scores matmul
        nc.vector.tensor_copy(qt[0][:, 0:512], ps_q0a)
        nc.scalar.copy(qt[0][:, 512:1024], ps_q0b)
        nc.vector.tensor_copy(kt[0][:, 0:512], ps_k0a[:, 0:512])
        nc.scalar.copy(kt[0][:, 512:1024], ps_k0a[:, 512:1024])
        nc.vector.tensor_copy(kt[0][:, 1024:1536], ps_k0b[:, 0:512])
        nc.scalar.copy(kt[0][:, 1536:2048], ps_k0b[:, 512:1024])
        nc.vector.tensor_copy(qt[1][:, 0:256], ps_q1a)
        nc.vector.tensor_copy(qt[1][:, 256:512], ps_q1b)

        # ---------------- fillers ----------------
        MM = 0.427  # us per 512-col matmul at full clock (cost bookkeeping)

        def q1cd_gen():
            # q1 quarters C/D through the freed attnv slots (xt resident)
            ps_c = ps_av.tile([P, 256], F32, name="ps_q1c", tag="av")
            for ci in range(8):
                mm(ps_c, wqk1_sb[:, ci, 0:P], xt[ci][:, 512:768],
                   start=(ci == 0), stop=(ci == 7), skip_group_check=True)
                yield 107
            ps_d = ps_av.tile([P, 256], F32, name="ps_q1d", tag="av")
            for ci in range(8):
                mm(ps_d, wqk1_sb[:, ci, 0:P], xt[ci][:, 768:1024],
                   start=(ci == 0), stop=(ci == 7), skip_group_check=True)
                yield 107
            nc.vector.tensor_copy(qt[1][:, 512:768], ps_c)
            nc.vector.tensor_copy(qt[1][:, 768:1024], ps_d)
            yield 0

        def k1_gen(quarter):
            ps = ps_sq.tile([P, 512], F32, name=f"ps_k1{quarter}", tag="sq")
            nk0 = quarter * 512
            for ci in range(8):
                lw = wqk1_sb[:, ci, P:2 * P]
                mm(ps, lw, xt[ci][:, nk0:nk0 + 512],
                   start=(ci == 0), stop=(ci == 7), skip_group_check=True)
                yield 213
            nc.vector.tensor_copy(kt[1][:, nk0:nk0 + 512], ps)
            yield 0

        def v_gen(t):
            # 2 kv-blocks (j = 2t, 2t+1) share one psum slot; one wide evac
            ps = ps_sq.tile([P, 2, DH], F32, name=f"ps_v{t}", tag="sq")
            for ci in range(8):
                for jj in range(2):
                    j = 2 * t + jj
                    # one start per psum BANK: jj=1's first matmul relies on
                    # the pending-zero left by jj=0's start
                    mm(ps[:, jj, :], xt[ci][:, j * P:(j + 1) * P],
                       wv_sb[:, ci, :],
                       start=(ci == 0 and jj == 0),
                       stop=(ci == 7 and jj == 1), skip_group_check=True)
                yield 214
            nc.vector.tensor_copy(
                v_sb[:, 2 * t:2 * t + 2, :, 0:D],
                ps.rearrange("p j (h d) -> p j h d", h=HPC))
            yield 0

        def proj0_gen(m):
            # two independent half-column units -> 2-wide through the sq slots
            for nh in range(2):
                ps = ps_sq.tile([P, 512], F32, name=f"ps_pj0_{m}_{nh}", tag="sq")
                mm(ps, attn_T[0][:, m, :],
                   wp_sb[0][:, nh * 512:(nh + 1) * 512],
                   start=True, stop=True, skip_group_check=True)
                yield 213
                fin = finp.tile([P, 512], BF16, name=f"fin0_{m}_{nh}", tag="fin")
                nc.vector.tensor_copy(fin, ps)
                nc.sync.dma_start(
                    out=outA[m * P:(m + 1) * P, nh * 512:(nh + 1) * 512],
                    in_=fin)
                yield 0

        # ---------------- attention pieces ----------------
        av_tiles = {}

        def alloc_av(h):
            av_tiles[h] = [ps_av.tile([P, 4, D + 1], F32, name=f"av{h}_{s}",
                                      tag="av") for s in range(2)]

        ets = {}

        def scores_j(h, j):
            pair, po = h // 2, D * (h % 2)
            ps = ps_sc.tile([P, NQ], F32, name=f"ps_s{h}_{j}", tag="sc")
            lw = kt[pair][po:po + D, j * P:(j + 1) * P]
            for nh in range(2):
                mm(ps[:, nh * 512:(nh + 1) * 512], lw,
                   qt[pair][po:po + D, nh * 512:(nh + 1) * 512],
                   start=True, stop=True, skip_group_check=True)
            et = ets_pool.tile([P, NQ], BF16, name=f"et{h}_{j}", tag="ets")
            nc.scalar.activation(out=et, in_=ps, func=Exp, scale=SCALE)
            ets[(h, j)] = et

        def attnv_j(h, j):
            et = ets[(h, j)]
            for qb in range(8):
                av = av_tiles[h][qb // 4]
                mm(av[:, qb % 4, :],
                   et[:, qb * P:(qb + 1) * P],
                   v_sb[:, j, h, :],
                   start=(j == 0 and qb % 4 == 0),
                   stop=(j == 15 and qb % 4 == 3),
                   skip_group_check=True)

        def norm_half(h, part, tail):
            # tail=False: DVE + Pool (ACT is mid-exp-stream); tail=True:
            # DVE + ACT (lower latency, stream over)
            pair, half = h // 2, h % 2
            av = av_tiles[h][part]
            nc.vector.reciprocal(rcp[:, h, part * 4:(part + 1) * 4], av[:, :, D])
            for i in range(4):
                qb = part * 4 + i
                dst = attn_pack[pair][:, qb, half * D:(half + 1) * D]
                if tail and i % 2 == 1:
                    nc.scalar.activation(out=dst, in_=av[:, i, 0:D], func=Copy,
                                         scale=rcp[:, h, qb:qb + 1])
                else:
                    nc.vector.tensor_scalar_mul(dst, av[:, i, 0:D],
                                                rcp[:, h, qb:qb + 1])

        def pull(gen, budget):
            # cost-aware: drain up to ~budget ns of emitted matmul work
            acc = 0
            while acc < budget:
                c = next(gen, None)
                if c is None:
                    return False
                acc += c
            return True

        # ---------------- head loops (ACT exp stream is the pacer) --------
        # Each head's 16 exps give ~17.1us of ACT; scores are ~6.8us of PE,
        # leaving ~640ns/iter of PE filler budget.
        # h0: v pairs 0..4 (j0..9) + q1 quarters C/D
        f = chain(v_gen(0), v_gen(1), q1cd_gen(), v_gen(2), v_gen(3), v_gen(4))
        for j in range(16):
            scores_j(0, j)
            pull(f, 640)
        for _ in f:
            pass

        # h1: k1a + k1b first (unblocks h2 scores), then v pairs 5, 6
        f = chain(*(k1_gen(qu) for qu in range(4)), *(v_gen(t) for t in (5, 6)))
        for j in range(16):
            scores_j(1, j)
            pull(f, 615)
        for _ in f:
            pass

        # h2: v pair 7 early + attnv(h0) iters 0..4, norm(h0)@5,
        #     attnv(h1) 6..11, norm(h1)@12, pair0 transpose @12
        A0 = [(0, 1), (1, 4), (4, 7), (7, 10), (10, 13), (13, 16)]
        A1 = [(0, 3), (3, 5), (5, 8), (8, 10), (10, 13), (13, 16)]
        alloc_av(0)
        fv = chain(v_gen(7))
        fp = chain(*(proj0_gen(m) for m in range(8)))

        def tp0(qb):
            tp = ps_av.tile([P, P], BF16, name=f"tp0_{qb}", tag="av")
            nc.tensor.transpose(tp, attn_pack[0][:, qb, :], identity)
            nc.vector.tensor_copy(attn_T[0][:, qb, :], tp)

        for j in range(16):
            scores_j(2, j)
            if j < 6:
                for jj in range(*A0[j]):
                    attnv_j(0, jj)
            elif j == 6:
                norm_half(0, 0, False)
                norm_half(0, 1, False)
                alloc_av(1)
            elif j < 13:
                for jj in range(*A1[j - 7]):
                    attnv_j(1, jj)
            elif j == 13:
                norm_half(1, 0, False)
                norm_half(1, 1, False)
            elif j >= 14:
                for qb in (2 * (j - 14), 2 * (j - 14) + 1):
                    tp0(qb)
            if j < 4:
                pull(fv, 430)

        # h3: attnv(h2) iters 0..7, norm(h2)@8, attnv(h3) j0..14 iters 8..15,
        #     rest of proj0 spread over all iters
        alloc_av(2)
        for j in range(16):
            scores_j(3, j)
            if j < 2:
                tp0(4 + 2 * j)
                tp0(5 + 2 * j)
            if j < 8:
                attnv_j(2, 2 * j)
                attnv_j(2, 2 * j + 1)
            else:
                if j == 8:
                    norm_half(2, 0, False)
                elif j == 9:
                    norm_half(2, 1, False)
                    alloc_av(3)
                if j >= 9:
                    for jj in range((j - 9) * 15 // 7, (j - 8) * 15 // 7):
                        attnv_j(3, jj)
            if 1 <= j <= 6:
                pull(fp, 520)
            elif j >= 10:
                pull(fp, 570)
        for _ in fp:
            pass

        # ---------------- tail ----------------
        attnv_j(3, 15)

        # Per-qb chain: normalize -> PE transpose -> evac -> project -> fin
        # -> DMA, with DVE/ACT/Pool round-robin so no single evac engine
        # serializes the drain.  proj psums 2-deep via the sc tag.
        av3 = av_tiles[3]
        nc.vector.reciprocal(rcp[:, 3, 0:4], av3[0][:, :, D])
        nc.vector.reciprocal(rcp[:, 3, 4:8], av3[1][:, :, D])

        def mul3(qb):
            dst = attn_pack[1][:, qb, D:2 * D]
            src_ = av3[qb // 4][:, qb % 4, 0:D]
            if qb % 2 == 0:
                nc.vector.tensor_scalar_mul(dst, src_, rcp[:, 3, qb:qb + 1])
            else:
                nc.scalar.activation(out=dst, in_=src_, func=Copy,
                                     scale=rcp[:, 3, qb:qb + 1])

        def tp1(qb):
            tp = ps_sq.tile([P, P], BF16, name=f"tp{qb}", tag="sq")
            nc.tensor.transpose(tp, attn_pack[1][:, qb, :], identity)
            if qb % 2 == 0:
                nc.vector.tensor_copy(attn_T[1][:, qb, :], tp)
            else:
                nc.scalar.copy(attn_T[1][:, qb, :], tp)

        # pair1 projection at quarter-column granularity: 32 independent
        # [128, 256] psum units spread across all three free pools (6 slots
        # in flight), single-engine fin per quarter (round-robin), one DMA
        # per m-block.
        qslots = [(ps_av, "av"), (ps_sc, "sc"), (ps_sq, "sq")]

        def proj1(m):
            fin = finp.tile([P, C], BF16, name=f"fin1_{m}", tag="fin")
            for qo in range(4):
                k = 4 * m + qo
                pool, tag = qslots[k % 3]
                ps = pool.tile([P, 256], F32, name=f"pj1_{m}_{qo}", tag=tag)
                mm(ps, attn_T[1][:, m, :],
                   wp_sb[1][:, qo * 256:(qo + 1) * 256],
                   start=True, stop=True, skip_group_check=True)
                dst = fin[:, qo * 256:(qo + 1) * 256]
                if k % 2 == 0:
                    nc.scalar.copy(dst, ps)
                else:
                    nc.vector.tensor_copy(dst, ps)
            nc.sync.dma_start(out=outB[m * P:(m + 1) * P, :], in_=fin)

        mul3(0)
        tp1(0)
        mul3(1)
        tp1(1)
        for qb in range(2, 8):
            mul3(qb)
            tp1(qb)
            proj1(qb - 2)
        proj1(6)
        proj1(7)


def _get_nc():
    if "nc" not in _CACHE:
        _CACHE["nc"] = _build()
    return _CACHE["nc"]


def kernel(x, wq, wk, wv, w_proj, b_proj):
    x = np.asarray(x, dtype=np.float32)
    wq = np.asarray(wq, dtype=np.float32)
    wk = np.asarray(wk, dtype=np.float32)
    wv = np.asarray(wv, dtype=np.float32)
    w_proj = np.asarray(w_proj, dtype=np.float32)
    b_proj = np.asarray(b_proj, dtype=np.float32)

    nc = _get_nc()
    in_maps = []
    for core in range(8):
        b, g = divmod(core, 4)
        s0 = g * DH
        p0 = slice(s0, s0 + P)            # pair0 rows (heads 4g, 4g+1)
        p1 = slice(s0 + P, s0 + 2 * P)    # pair1 rows
        sl = slice(s0, s0 + DH)
        in_maps.append({
            "xT": np.ascontiguousarray(x[b].T).astype(_BF),
            "wqk0": np.ascontiguousarray(
                np.hstack([wq[p0, :].T, wk[p0, :].T])).astype(_BF),
            "wqk1": np.ascontiguousarray(
                np.hstack([wq[p1, :].T, wk[p1, :].T])).astype(_BF),
            "wvT": np.ascontiguousarray(wv[sl, :].T).astype(_BF),
            "wpT": np.ascontiguousarray(w_proj[:, sl].T).astype(_BF),
        })

    res = run_bass_kernel_spmd(nc, in_maps, core_ids=list(range(8)),
                               trace=bool(int(os.environ.get("KERNEL_TRACE", "0"))))
    _CACHE["last_results"] = res
    acc = [np.zeros((NQ, C), np.float32) for _ in range(2)]
    for core in range(8):
        b = core // 4
        acc[b] += res.results[core]["outA"].astype(np.float32)
        acc[b] += res.results[core]["outB"].astype(np.float32)
    full = np.stack(acc)
    full += b_proj[None, None, :]
    return full.astype(np.float32)


# revision 51
# speedup vs baseline: 1.0076x; 1.0024x over previous
"""Cross-attention kernel for Trainium2, 8-core SPMD (v3: bf16 + transposed attnv).

Problem (all fp32):
  x [2, 2048, 1024]; wq/wk/wv/w_proj [1024, 1024]; b_proj [1024]
  q = x[:, :1024] @ wq.T   (16 heads x 64)
  k, v = x @ wk.T, x @ wv.T
  out = softmax(q k^T / 8) v  -> proj + bias  -> [2, 1024, 1024]

Sharding: 8 cores = 2 (batch) x 4 (head-groups of 4 heads = 2 pairs of 2).
Each core emits TWO bf16 partials (one per head-pair); host upcasts, sums
the 16 partials per batch and adds the bias.

Design (matmul cost = out-cols x 0.4167ns x cpr; bf16 cpr=1 at any width,
fp32r cpr=4 below 256 cols):
  - x/weights stream in as bf16 (half DMA bytes); q/k kept fp32r so scores
    stay high precision; exp output, v, attn, proj all bf16.
  - attnv is transposed: stationary = exp tile [128kv, 128q], moving =
    v [128kv, 65] -> psum [q-block, 65].  8320 cols/head vs 16384, and the
    ones-column denominator lands per-PARTITION, so normalization is a
    cheap DVE tensor_scalar multiply (no PE broadcast matmuls).
  - normalized attn for a head-pair is packed [128q, 128dd], transposed
    (pair0: DMA-xbar mid-kernel; pair1: PE transpose in the tail where
    PSUM is free) and projected with a full-128 contraction.
  - PSUM (8 banks): scores [128, 1024] x2 (4) + attnv 2 x [128, 4, 65]
    (2) + one [128, 1024] rotating "seq" slot (2) for k1a/k1b/v_j/proj0.
    q pair1 runs inside stage A (its own psum there is the seq slot's
    first user).  Tail projection alternates the sc and seq tags for
    2-deep pipelining.
  - The exp stream (64 x [128, 1024], ~1.07us each) is the pacing engine;
    PE in-loop work is levelled across the 4 head loops so ACT never
    starves: h0 carries v j0..11, h1 carries k1 + v j12..15 + attnv(h0),
    h2 carries attnv(h1), h3 carries attnv(h2) + attnv(h3) + proj0.
"""

import os
import numpy as np
import ml_dtypes

import concourse.bacc as bacc
import concourse.bass as bass
import concourse.tile as tile
import concourse.mybir as mybir
from concourse.bass_utils import run_bass_kernel_spmd
from concourse.masks import make_identity

F32 = mybir.dt.float32
F32R = mybir.dt.float32r
BF16 = mybir.dt.bfloat16

C = 1024          # model dim
N = 2048          # kv tokens
NQ = 1024         # query tokens
HPC = 4           # heads per core
D = 64            # head dim
DH = HPC * D      # per-core slice of C (256)
SCALE = D ** -0.5
P = 128

_CACHE: dict = {}
_BF = ml_dtypes.bfloat16


def _build():
    nc = bacc.Bacc("TRN2", target_bir_lowering=False, debug=False, num_devices=8)

    xT = nc.dram_tensor("xT", [C, N], BF16, kind="ExternalInput").ap()
    # wqk{p} = hstack(wq[pair p slice].T, wk[pair p slice].T)  [C, 256]
    wqk0 = nc.dram_tensor("wqk0", [C, 2 * P], BF16, kind="ExternalInput").ap()
    wqk1 = nc.dram_tensor("wqk1", [C, 2 * P], BF16, kind="ExternalInput").ap()
    wvT = nc.dram_tensor("wvT", [C, DH], BF16, kind="ExternalInput").ap()
    wpT = nc.dram_tensor("wpT", [DH, C], BF16, kind="ExternalInput").ap()
    outA = nc.dram_tensor("outA", [NQ, C], BF16, kind="ExternalOutput").ap()
    outB = nc.dram_tensor("outB", [NQ, C], BF16, kind="ExternalOutput").ap()

    with tile.TileContext(nc) as tc, \
            nc.allow_low_precision(reason="bf16 pipeline within 2e-2 tolerance"):
        _emit(tc, xT, wqk0, wqk1, wvT, wpT, outA, outB)

    nc.compile()
    return nc


def _emit(tc, xT, wqk0, wqk1, wvT, wpT, outA, outB):
    nc = tc.nc
    mm = nc.tensor.matmul
    Exp = mybir.ActivationFunctionType.Exp
    Copy = mybir.ActivationFunctionType.Copy

    from contextlib import ExitStack
    from itertools import chain

    with ExitStack() as ctx:
        singles = ctx.enter_context(tc.tile_pool(name="singles", bufs=1))
        ets_pool = ctx.enter_context(tc.tile_pool(name="ets", bufs=32))
        finp = ctx.enter_context(tc.tile_pool(name="finp", bufs=8))
        ps_sc = ctx.enter_context(tc.tile_pool(name="ps_sc", bufs=2, space="PSUM"))
        ps_av = ctx.enter_context(tc.tile_pool(name="ps_av", bufs=2, space="PSUM"))
        ps_sq = ctx.enter_context(tc.tile_pool(name="ps_sq", bufs=2, space="PSUM"))

        # ---------------- input DMAs (one ordered SP/HWDGE stream) --------
        # Per chunk: wqk0_ci, wqk1_ci, x_ci  (stage A consumes q0/k0/q1 per
        # chunk as it lands); then wv, wp (needed from ~h0/h3).
        xt = [singles.tile([P, N], BF16, name=f"xt{ci}", tag=f"xt{ci}")
              for ci in range(8)]
        wqk0_sb = singles.tile([P, 8, 2 * P], BF16, name="wqk0_sb", tag="wqk0")
        wqk1_sb = singles.tile([P, 8, 2 * P], BF16, name="wqk1_sb", tag="wqk1")
        wv_sb = singles.tile([P, 8, DH], BF16, name="wv_sb", tag="wv")
        wp_sb = [singles.tile([P, C], BF16, name=f"wp{p}", tag=f"wp{p}")
                 for p in range(2)]

        wqk0_src = wqk0.rearrange("(a p) d -> p a d", p=P)
        wqk1_src = wqk1.rearrange("(a p) d -> p a d", p=P)
        wv_src = wvT.rearrange("(a p) d -> p a d", p=P)

        for ci in range(8):
            nc.sync.dma_start(out=wqk0_sb[:, ci, :], in_=wqk0_src[:, ci, :])
            if ci >= 6:
                nc.sync.dma_start(out=xt[ci][:, 0:1024],
                                  in_=xT[ci * P:(ci + 1) * P, 0:1024])
                nc.sync.dma_start(out=xt[ci][:, 1024:2048],
                                  in_=xT[ci * P:(ci + 1) * P, 1024:2048])
            else:
                nc.sync.dma_start(out=xt[ci], in_=xT[ci * P:(ci + 1) * P, :])
        for ci in range(8):
            nc.sync.dma_start(out=wqk1_sb[:, ci, :], in_=wqk1_src[:, ci, :])
            nc.sync.dma_start(out=wv_sb[:, ci, :], in_=wv_src[:, ci, :])
        for p in range(2):
            nc.sync.dma_start(out=wp_sb[p], in_=wpT[p * P:(p + 1) * P, :])

        # ---------------- small consts ----------------
        identity = singles.tile([P, P], BF16, name="identity", tag="ident")
        make_identity(nc, identity)

        # Pre-trigger the exp table load while DMAs stream.
        dmt = singles.tile([1, 1], BF16, name="dmt", tag="dmt")
        nc.scalar.activation(out=dmt, in_=identity[0:1, 0:1], func=Exp, scale=1.0)

        # ---------------- persistent SBUF ----------------
        qt = [singles.tile([P, NQ], F32R, name=f"qt{p}", tag=f"qt{p}")
              for p in range(2)]
        kt = [singles.tile([P, N], F32R, name=f"kt{p}", tag=f"kt{p}")
              for p in range(2)]
        v_sb = singles.tile([P, 16, HPC, D + 1], BF16, name="v_sb", tag="v_sb")
        nc.gpsimd.memset(v_sb[:, :, :, D:D + 1], 1.0)

        attn_pack = [singles.tile([P, 8, P], BF16, name=f"apk{p}", tag=f"apk{p}")
                     for p in range(2)]
        attn_T = [singles.tile([P, 8, P], BF16, name=f"atT{p}", tag=f"atT{p}")
                  for p in range(2)]
        rcp = singles.tile([P, HPC, 8], F32, name="rcp", tag="rcp")

        # ---------------- stage A: q pair0 + k pair0 + q1 quarters --------
        # 8 mm per chunk vs ~1.6us chunk arrival: roughly DMA-paced.  q1 is
        # split into four [128, 256] quarter-psums so two of them fit the
        # (otherwise idle) 1-bank attnv slots during stage A; the other two
        # run as the first h0 fillers.
        ps_q0a = ps_sq.tile([P, 512], F32, name="ps_q0a", tag="sq")
        ps_q0b = ps_sq.tile([P, 512], F32, name="ps_q0b", tag="sq")
        ps_k0a = ps_sc.tile([P, NQ], F32, name="ps_k0a", tag="sc")
        ps_k0b = ps_sc.tile([P, NQ], F32, name="ps_k0b", tag="sc")
        ps_q1a = ps_av.tile([P, 256], F32, name="ps_q1a", tag="av")
        ps_q1b = ps_av.tile([P, 256], F32, name="ps_q1b", tag="av")
        for ci in range(8):
            lw_q0 = wqk0_sb[:, ci, 0:P]
            lw_k0 = wqk0_sb[:, ci, P:2 * P]
            lw_q1 = wqk1_sb[:, ci, 0:P]
            st = dict(start=(ci == 0), stop=(ci == 7), skip_group_check=True)
            mm(ps_q0a, lw_q0, xt[ci][:, 0:512], **st)
            mm(ps_k0a[:, 0:512], lw_k0, xt[ci][:, 0:512], **st)
            mm(ps_q0b, lw_q0, xt[ci][:, 512:1024], **st)
            mm(ps_k0a[:, 512:1024], lw_k0, xt[ci][:, 512:1024], **st)
            mm(ps_k0b[:, 0:512], lw_k0, xt[ci][:, 1024:1536], **st)
            mm(ps_k0b[:, 512:1024], lw_k0, xt[ci][:, 1536:2048], **st)
            mm(ps_q1a, lw_q1, xt[ci][:, 0:256], **st)
            mm(ps_q1b, lw_q1, xt[ci][:, 256:512], **st)
        # evacs split across DVE+ACT+Pool (all idle pre-stream) to shorten
        # the serial path to the first scores matmul
        nc.vector.tensor_copy(qt[0][:, 0:512], ps_q0a)
        nc.scalar.copy(qt[0][:, 512:1024], ps_q0b)
        nc.vector.tensor_copy(kt[0][:, 0:512], ps_k0a[:, 0:512])
        nc.scalar.copy(kt[0][:, 512:1024], ps_k0a[:, 512:1024])
        nc.vector.tensor_copy(kt[0][:, 1024:1536], ps_k0b[:, 0:512])
        nc.scalar.copy(kt[0][:, 1536:2048], ps_k0b[:, 512:1024])
        nc.vector.tensor_copy(qt[1][:, 0:256], ps_q1a)
        nc.vector.tensor_copy(qt[1][:, 256:512], ps_q1b)

        # ---------------- fillers ----------------
        MM = 0.427  # us per 512-col matmul at full clock (cost bookkeeping)

        def q1cd_gen():
            # q1 quarters C/D through the freed attnv slots (xt resident)
            ps_c = ps_av.tile([P, 256], F32, name="ps_q1c", tag="av")
            for ci in range(8):
                mm(ps_c, wqk1_sb[:, ci, 0:P], xt[ci][:, 512:768],
                   start=(ci == 0), stop=(ci == 7), skip_group_check=True)
                yield 107
            ps_d = ps_av.tile([P, 256], F32, name="ps_q1d", tag="av")
            for ci in range(8):
                mm(ps_d, wqk1_sb[:, ci, 0:P], xt[ci][:, 768:1024],
                   start=(ci == 0), stop=(ci == 7), skip_group_check=True)
                yield 107
            nc.vector.tensor_copy(qt[1][:, 512:768], ps_c)
            nc.vector.tensor_copy(qt[1][:, 768:1024], ps_d)
            yield 0

        def k1_gen(quarter):
            ps = ps_sq.tile([P, 512], F32, name=f"ps_k1{quarter}", tag="sq")
            nk0 = quarter * 512
            for ci in range(8):
                lw = wqk1_sb[:, ci, P:2 * P]
                mm(ps, lw, xt[ci][:, nk0:nk0 + 512],
                   start=(ci == 0), stop=(ci == 7), skip_group_check=True)
                yield 213
            nc.vector.tensor_copy(kt[1][:, nk0:nk0 + 512], ps)
            yield 0

        def v_gen(t):
            # 2 kv-blocks (j = 2t, 2t+1) share one psum slot; one wide evac
            ps = ps_sq.tile([P, 2, DH], F32, name=f"ps_v{t}", tag="sq")
            for ci in range(8):
                for jj in range(2):
                    j = 2 * t + jj
                    # one start per psum BANK: jj=1's first matmul relies on
                    # the pending-zero left by jj=0's start
                    mm(ps[:, jj, :], xt[ci][:, j * P:(j + 1) * P],
                       wv_sb[:, ci, :],
                       start=(ci == 0 and jj == 0),
                       stop=(ci == 7 and jj == 1), skip_group_check=True)
                yield 214
            nc.vector.tensor_copy(
                v_sb[:, 2 * t:2 * t + 2, :, 0:D],
                ps.rearrange("p j (h d) -> p j h d", h=HPC))
            yield 0

        def proj0_gen(m):
            # two independent half-column units -> 2-wide through the sq slots
            for nh in range(2):
                ps = ps_sq.tile([P, 512], F32, name=f"ps_pj0_{m}_{nh}", tag="sq")
                mm(ps, attn_T[0][:, m, :],
                   wp_sb[0][:, nh * 512:(nh + 1) * 512],
                   start=True, stop=True, skip_group_check=True)
                yield 213
                fin = finp.tile([P, 512], BF16, name=f"fin0_{m}_{nh}", tag="fin")
                nc.vector.tensor_copy(fin, ps)
                nc.sync.dma_start(
                    out=outA[m * P:(m + 1) * P, nh * 512:(nh + 1) * 512],
                    in_=fin)
                yield 0

        # ---------------- attention pieces ----------------
        av_tiles = {}

        def alloc_av(h):
            av_tiles[h] = [ps_av.tile([P, 4, D + 1], F32, name=f"av{h}_{s}",
                                      tag="av") for s in range(2)]

        ets = {}

        def scores_j(h, j):
            pair, po = h // 2, D * (h % 2)
            ps = ps_sc.tile([P, NQ], F32, name=f"ps_s{h}_{j}", tag="sc")
            lw = kt[pair][po:po + D, j * P:(j + 1) * P]
            for nh in range(2):
                mm(ps[:, nh * 512:(nh + 1) * 512], lw,
                   qt[pair][po:po + D, nh * 512:(nh + 1) * 512],
                   start=True, stop=True, skip_group_check=True)
            et = ets_pool.tile([P, NQ], BF16, name=f"et{h}_{j}", tag="ets")
            nc.scalar.activation(out=et, in_=ps, func=Exp, scale=SCALE)
            ets[(h, j)] = et

        def attnv_j(h, j):
            et = ets[(h, j)]
            for qb in range(8):
                av = av_tiles[h][qb // 4]
                mm(av[:, qb % 4, :],
                   et[:, qb * P:(qb + 1) * P],
                   v_sb[:, j, h, :],
                   start=(j == 0 and qb % 4 == 0),
                   stop=(j == 15 and qb % 4 == 3),
                   skip_group_check=True)

        def norm_half(h, part, tail):
            # tail=False: DVE + Pool (ACT is mid-exp-stream); tail=True:
            # DVE + ACT (lower latency, stream over)
            pair, half = h // 2, h % 2
            av = av_tiles[h][part]
            nc.vector.reciprocal(rcp[:, h, part * 4:(part + 1) * 4], av[:, :, D])
            for i in range(4):
                qb = part * 4 + i
                dst = attn_pack[pair][:, qb, half * D:(half + 1) * D]
                if tail and i % 2 == 1:
                    nc.scalar.activation(out=dst, in_=av[:, i, 0:D], func=Copy,
                                         scale=rcp[:, h, qb:qb + 1])
                else:
                    nc.vector.tensor_scalar_mul(dst, av[:, i, 0:D],
                                                rcp[:, h, qb:qb + 1])

        def pull(gen, budget):
            # cost-aware: drain up to ~budget ns of emitted matmul work
            acc = 0
            while acc < budget:
                c = next(gen, None)
                if c is None:
                    return False
                acc += c
            return True

        # ---------------- head loops (ACT exp stream is the pacer) --------
        # Each head's 16 exps give ~17.1us of ACT; scores are ~6.8us of PE,
        # leaving ~640ns/iter of PE filler budget.
        # h0: v pairs 0..4 (j0..9) + q1 quarters C/D
        f = chain(v_gen(0), v_gen(1), q1cd_gen(), v_gen(2), v_gen(3), v_gen(4))
        for j in range(16):
            scores_j(0, j)
            pull(f, 640)
        for _ in f:
            pass

        # h1: k1a + k1b first (unblocks h2 scores), then v pairs 5, 6
        f = chain(*(k1_gen(qu) for qu in range(4)), *(v_gen(t) for t in (5, 6)))
        for j in range(16):
            scores_j(1, j)
            pull(f, 615)
        for _ in f:
            pass

        # h2: v pair 7 early + attnv(h0) iters 0..4, norm(h0)@5,
        #     attnv(h1) 6..11, norm(h1)@12, pair0 transpose @12
        A0 = [(0, 1), (1, 4), (4, 7), (7, 10), (10, 13), (13, 16)]
        A1 = [(0, 3), (3, 5), (5, 8), (8, 10), (10, 13), (13, 16)]
        alloc_av(0)
        fv = chain(v_gen(7))
        fp = chain(*(proj0_gen(m) for m in range(8)))

        def tp0(qb):
            tp = ps_av.tile([P, P], BF16, name=f"tp0_{qb}", tag="av")
            nc.tensor.transpose(tp, attn_pack[0][:, qb, :], identity)
            nc.vector.tensor_copy(attn_T[0][:, qb, :], tp)

        for j in range(16):
            scores_j(2, j)
            if j < 6:
                for jj in range(*A0[j]):
                    attnv_j(0, jj)
            elif j == 6:
                norm_half(0, 0, False)
                norm_half(0, 1, False)
                alloc_av(1)
            elif j < 13:
                for jj in range(*A1[j - 7]):
                    attnv_j(1, jj)
            elif j == 13:
                norm_half(1, 0, False)
                norm_half(1, 1, False)
            elif j >= 14:
                for qb in (2 * (j - 14), 2 * (j - 14) + 1):
                    tp0(qb)
            if j < 4:
                pull(fv, 430)

        # h3: attnv(h2) iters 0..7, norm(h2)@8, attnv(h3) j0..14 iters 8..15,
        #     rest of proj0 spread over all iters
        alloc_av(2)
        for j in range(16):
            scores_j(3, j)
            if j < 2:
                tp0(4 + 2 * j)
                tp0(5 + 2 * j)
            if j < 8:
                attnv_j(2, 2 * j)
                attnv_j(2, 2 * j + 1)
            else:
                if j == 8:
                    norm_half(2, 0, False)
                elif j == 9:
                    norm_half(2, 1, False)
                    alloc_av(3)
                if j >= 9:
                    for jj in range((j - 9) * 15 // 7, (j - 8) * 15 // 7):
                        attnv_j(3, jj)
            if 1 <= j <= 6:
                pull(fp, 520)
            elif j >= 10:
                pull(fp, 570)
        for _ in fp:
            pass

        # ---------------- tail ----------------
        attnv_j(3, 15)

        # Per-qb chain: normalize -> PE transpose -> evac -> project -> fin
        # -> DMA, with DVE/ACT/Pool round-robin so no single evac engine
        # serializes the drain.  proj psums 2-deep via the sc tag.
        av3 = av_tiles[3]
        nc.vector.reciprocal(rcp[:, 3, 0:4], av3[0][:, :, D])
        nc.vector.reciprocal(rcp[:, 3, 4:8], av3[1][:, :, D])

        def mul3(qb):
            dst = attn_pack[1][:, qb, D:2 * D]
            src_ = av3[qb // 4][:, qb % 4, 0:D]
            if qb % 2 == 0:
                nc.vector.tensor_scalar_mul(dst, src_, rcp[:, 3, qb:qb + 1])
            else:
                nc.scalar.activation(out=dst, in_=src_, func=Copy,
                                     scale=rcp[:, 3, qb:qb + 1])

        def tp1(qb):
            tp = ps_sq.tile([P, P], BF16, name=f"tp{qb}", tag="sq")
            nc.tensor.transpose(tp, attn_pack[1][:, qb, :], identity)
            if qb % 2 == 0:
                nc.vector.tensor_copy(attn_T[1][:, qb, :], tp)
            else:
                nc.scalar.copy(attn_T[1][:, qb, :], tp)

        # pair1 projection at full-block granularity: both nh matmuls into
        # one 2-bank sc tile, then a SINGLE fin evacuation per m-block on
        # alternating engines (ACT/DVE) -- fewest per-op overheads; the two
        # engines leapfrog so consecutive blocks' evacuations overlap.

        def proj1(m):
            fin = finp.tile([P, C], BF16, name=f"fin1_{m}", tag="fin")
            ps = ps_sc.tile([P, NQ], F32, name=f"pj1_{m}", tag="sc")
            for nh in range(2):
                mm(ps[:, nh * 512:(nh + 1) * 512],
                   attn_T[1][:, m, :],
                   wp_sb[1][:, nh * 512:(nh + 1) * 512],
                   start=True, stop=True, skip_group_check=True)
            if m % 2 == 0:
                nc.scalar.copy(fin, ps)
            else:
                nc.vector.tensor_copy(fin, ps)
            nc.sync.dma_start(out=outB[m * P:(m + 1) * P, :], in_=fin)

        mul3(0)
        tp1(0)
        mul3(1)
        tp1(1)
        for qb in range(2, 8):
            mul3(qb)
            tp1(qb)
            proj1(qb - 2)
        proj1(6)
        proj1(7)


def _get_nc():
    if "nc" not in _CACHE:
        _CACHE["nc"] = _build()
    return _CACHE["nc"]


def kernel(x, wq, wk, wv, w_proj, b_proj):
    x = np.asarray(x, dtype=np.float32)
    wq = np.asarray(wq, dtype=np.float32)
    wk = np.asarray(wk, dtype=np.float32)
    wv = np.asarray(wv, dtype=np.float32)
    w_proj = np.asarray(w_proj, dtype=np.float32)
    b_proj = np.asarray(b_proj, dtype=np.float32)

    nc = _get_nc()
    in_maps = []
    for core in range(8):
        b, g = divmod(core, 4)
        s0 = g * DH
        p0 = slice(s0, s0 + P)            # pair0 rows (heads 4g, 4g+1)
        p1 = slice(s0 + P, s0 + 2 * P)    # pair1 rows
        sl = slice(s0, s0 + DH)
        in_maps.append({
            "xT": np.ascontiguousarray(x[b].T).astype(_BF),
            "wqk0": np.ascontiguousarray(
                np.hstack([wq[p0, :].T, wk[p0, :].T])).astype(_BF),
            "wqk1": np.ascontiguousarray(
                np.hstack([wq[p1, :].T, wk[p1, :].T])).astype(_BF),
            "wvT": np.ascontiguousarray(wv[sl, :].T).astype(_BF),
            "wpT": np.ascontiguousarray(w_proj[:, sl].T).astype(_BF),
        })

    res = run_bass_kernel_spmd(nc, in_maps, core_ids=list(range(8)),
                               trace=bool(int(os.environ.get("KERNEL_TRACE", "0"))))
    _CACHE["last_results"] = res
    acc = [np.zeros((NQ, C), np.float32) for _ in range(2)]
    for core in range(8):
        b = core // 4
        acc[b] += res.results[core]["outA"].astype(np.float32)
        acc[b] += res.results[core]["outB"].astype(np.float32)
    full = np.stack(acc)
    full += b_proj[None, None, :]
    return full.astype(np.float32)


# revision 55
# speedup vs baseline: 1.0114x; 1.0038x over previous
"""Cross-attention kernel for Trainium2, 8-core SPMD (v3: bf16 + transposed attnv).

Problem (all fp32):
  x [2, 2048, 1024]; wq/wk/wv/w_proj [1024, 1024]; b_proj [1024]
  q = x[:, :1024] @ wq.T   (16 heads x 64)
  k, v = x @ wk.T, x @ wv.T
  out = softmax(q k^T / 8) v  -> proj + bias  -> [2, 1024, 1024]

Sharding: 8 cores = 2 (batch) x 4 (head-groups of 4 heads = 2 pairs of 2).
Each core emits TWO bf16 partials (one per head-pair); host upcasts, sums
the 16 partials per batch and adds the bias.

Design (matmul cost = out-cols x 0.4167ns x cpr; bf16 cpr=1 at any width,
fp32r cpr=4 below 256 cols):
  - x/weights stream in as bf16 (half DMA bytes); q/k kept fp32r so scores
    stay high precision; exp output, v, attn, proj all bf16.
  - attnv is transposed: stationary = exp tile [128kv, 128q], moving =
    v [128kv, 65] -> psum [q-block, 65].  8320 cols/head vs 16384, and the
    ones-column denominator lands per-PARTITION, so normalization is a
    cheap DVE tensor_scalar multiply (no PE broadcast matmuls).
  - normalized attn for a head-pair is packed [128q, 128dd], transposed
    (pair0: DMA-xbar mid-kernel; pair1: PE transpose in the tail where
    PSUM is free) and projected with a full-128 contraction.
  - PSUM (8 banks): scores [128, 1024] x2 (4) + attnv 2 x [128, 4, 65]
    (2) + one [128, 1024] rotating "seq" slot (2) for k1a/k1b/v_j/proj0.
    q pair1 runs inside stage A (its own psum there is the seq slot's
    first user).  Tail projection alternates the sc and seq tags for
    2-deep pipelining.
  - The exp stream (64 x [128, 1024], ~1.07us each) is the pacing engine;
    PE in-loop work is levelled across the 4 head loops so ACT never
    starves: h0 carries v j0..11, h1 carries k1 + v j12..15 + attnv(h0),
    h2 carries attnv(h1), h3 carries attnv(h2) + attnv(h3) + proj0.
"""

import os
import numpy as np
import ml_dtypes

import concourse.bacc as bacc
import concourse.bass as bass
import concourse.tile as tile
import concourse.mybir as mybir
from concourse.bass_utils import run_bass_kernel_spmd
from concourse.masks import make_identity

F32 = mybir.dt.float32
F32R = mybir.dt.float32r
BF16 = mybir.dt.bfloat16

C = 1024          # model dim
N = 2048          # kv tokens
NQ = 1024         # query tokens
HPC = 4           # heads per core
D = 64            # head dim
DH = HPC * D      # per-core slice of C (256)
SCALE = D ** -0.5
P = 128

_CACHE: dict = {}
_BF = ml_dtypes.bfloat16


def _build():
    nc = bacc.Bacc("TRN2", target_bir_lowering=False, debug=False, num_devices=8)

    xT = nc.dram_tensor("xT", [C, N], BF16, kind="ExternalInput").ap()
    # wqk{p} = hstack(wq[pair p slice].T, wk[pair p slice].T)  [C, 256]
    wqk0 = nc.dram_tensor("wqk0", [C, 2 * P], BF16, kind="ExternalInput").ap()
    wqk1 = nc.dram_tensor("wqk1", [C, 2 * P], BF16, kind="ExternalInput").ap()
    wvT = nc.dram_tensor("wvT", [C, DH], BF16, kind="ExternalInput").ap()
    wpT = nc.dram_tensor("wpT", [DH, C], BF16, kind="ExternalInput").ap()
    outA = nc.dram_tensor("outA", [NQ, C], BF16, kind="ExternalOutput").ap()
    outB = nc.dram_tensor("outB", [NQ, C], BF16, kind="ExternalOutput").ap()

    with tile.TileContext(nc) as tc, \
            nc.allow_low_precision(reason="bf16 pipeline within 2e-2 tolerance"):
        _emit(tc, xT, wqk0, wqk1, wvT, wpT, outA, outB)

    nc.compile()
    return nc


def _emit(tc, xT, wqk0, wqk1, wvT, wpT, outA, outB):
    nc = tc.nc
    mm = nc.tensor.matmul
    Exp = mybir.ActivationFunctionType.Exp
    Copy = mybir.ActivationFunctionType.Copy

    from contextlib import ExitStack
    from itertools import chain

    with ExitStack() as ctx:
        singles = ctx.enter_context(tc.tile_pool(name="singles", bufs=1))
        ets_pool = ctx.enter_context(tc.tile_pool(name="ets", bufs=32))
        finp = ctx.enter_context(tc.tile_pool(name="finp", bufs=8))
        ps_sc = ctx.enter_context(tc.tile_pool(name="ps_sc", bufs=2, space="PSUM"))
        ps_av = ctx.enter_context(tc.tile_pool(name="ps_av", bufs=2, space="PSUM"))
        ps_sq = ctx.enter_context(tc.tile_pool(name="ps_sq", bufs=2, space="PSUM"))

        # ---------------- input DMAs (one ordered SP/HWDGE stream) --------
        # Per chunk: wqk0_ci, wqk1_ci, x_ci  (stage A consumes q0/k0/q1 per
        # chunk as it lands); then wv, wp (needed from ~h0/h3).
        xt = [singles.tile([P, N], BF16, name=f"xt{ci}", tag=f"xt{ci}")
              for ci in range(8)]
        wqk0_sb = singles.tile([P, 8, 2 * P], BF16, name="wqk0_sb", tag="wqk0")
        wqk1_sb = singles.tile([P, 8, 2 * P], BF16, name="wqk1_sb", tag="wqk1")
        wv_sb = singles.tile([P, 8, DH], BF16, name="wv_sb", tag="wv")
        wp_sb = [singles.tile([P, C], BF16, name=f"wp{p}", tag=f"wp{p}")
                 for p in range(2)]

        wqk0_src = wqk0.rearrange("(a p) d -> p a d", p=P)
        wqk1_src = wqk1.rearrange("(a p) d -> p a d", p=P)
        wv_src = wvT.rearrange("(a p) d -> p a d", p=P)

        for ci in range(8):
            nc.sync.dma_start(out=wqk0_sb[:, ci, :], in_=wqk0_src[:, ci, :])
            if ci >= 6:
                nc.sync.dma_start(out=xt[ci][:, 0:1024],
                                  in_=xT[ci * P:(ci + 1) * P, 0:1024])
                nc.sync.dma_start(out=xt[ci][:, 1024:2048],
                                  in_=xT[ci * P:(ci + 1) * P, 1024:2048])
            else:
                nc.sync.dma_start(out=xt[ci], in_=xT[ci * P:(ci + 1) * P, :])
        for ci in range(8):
            nc.sync.dma_start(out=wqk1_sb[:, ci, :], in_=wqk1_src[:, ci, :])
            nc.sync.dma_start(out=wv_sb[:, ci, :], in_=wv_src[:, ci, :])
        for p in range(2):
            nc.sync.dma_start(out=wp_sb[p], in_=wpT[p * P:(p + 1) * P, :])

        # ---------------- small consts ----------------
        identity = singles.tile([P, P], BF16, name="identity", tag="ident")
        make_identity(nc, identity)

        # Pre-trigger the exp table load while DMAs stream.
        dmt = singles.tile([1, 1], BF16, name="dmt", tag="dmt")
        nc.scalar.activation(out=dmt, in_=identity[0:1, 0:1], func=Exp, scale=1.0)

        # ---------------- persistent SBUF ----------------
        qt = [singles.tile([P, NQ], F32R, name=f"qt{p}", tag=f"qt{p}")
              for p in range(2)]
        kt = [singles.tile([P, N], F32R, name=f"kt{p}", tag=f"kt{p}")
              for p in range(2)]
        v_sb = singles.tile([P, 16, HPC, D + 1], BF16, name="v_sb", tag="v_sb")
        nc.gpsimd.memset(v_sb[:, :, :, D:D + 1], 1.0)

        attn_pack = [singles.tile([P, 8, P], BF16, name=f"apk{p}", tag=f"apk{p}")
                     for p in range(2)]
        attn_T = [singles.tile([P, 8, P], BF16, name=f"atT{p}", tag=f"atT{p}")
                  for p in range(2)]
        rcp = singles.tile([P, HPC, 8], F32, name="rcp", tag="rcp")

        # ---------------- stage A: q pair0 + k pair0 + q1 quarters --------
        # 8 mm per chunk vs ~1.6us chunk arrival: roughly DMA-paced.  q1 is
        # split into four [128, 256] quarter-psums so two of them fit the
        # (otherwise idle) 1-bank attnv slots during stage A; the other two
        # run as the first h0 fillers.
        ps_q0a = ps_sq.tile([P, 512], F32, name="ps_q0a", tag="sq")
        ps_q0b = ps_sq.tile([P, 512], F32, name="ps_q0b", tag="sq")
        ps_k0a = ps_sc.tile([P, NQ], F32, name="ps_k0a", tag="sc")
        ps_k0b = ps_sc.tile([P, NQ], F32, name="ps_k0b", tag="sc")
        ps_q1a = ps_av.tile([P, 256], F32, name="ps_q1a", tag="av")
        ps_q1b = ps_av.tile([P, 256], F32, name="ps_q1b", tag="av")
        for ci in range(8):
            lw_q0 = wqk0_sb[:, ci, 0:P]
            lw_k0 = wqk0_sb[:, ci, P:2 * P]
            lw_q1 = wqk1_sb[:, ci, 0:P]
            st = dict(start=(ci == 0), stop=(ci == 7), skip_group_check=True)
            mm(ps_q0a, lw_q0, xt[ci][:, 0:512], **st)
            mm(ps_k0a[:, 0:512], lw_k0, xt[ci][:, 0:512], **st)
            mm(ps_q0b, lw_q0, xt[ci][:, 512:1024], **st)
            mm(ps_k0a[:, 512:1024], lw_k0, xt[ci][:, 512:1024], **st)
            mm(ps_k0b[:, 0:512], lw_k0, xt[ci][:, 1024:1536], **st)
            mm(ps_k0b[:, 512:1024], lw_k0, xt[ci][:, 1536:2048], **st)
            mm(ps_q1a, lw_q1, xt[ci][:, 0:256], **st)
            mm(ps_q1b, lw_q1, xt[ci][:, 256:512], **st)
        # evacs split across DVE+ACT+Pool (all idle pre-stream) to shorten
        # the serial path to the first scores matmul
        nc.vector.tensor_copy(qt[0][:, 0:512], ps_q0a)
        nc.scalar.copy(qt[0][:, 512:1024], ps_q0b)
        nc.vector.tensor_copy(kt[0][:, 0:512], ps_k0a[:, 0:512])
        nc.scalar.copy(kt[0][:, 512:1024], ps_k0a[:, 512:1024])
        nc.vector.tensor_copy(kt[0][:, 1024:1536], ps_k0b[:, 0:512])
        nc.scalar.copy(kt[0][:, 1536:2048], ps_k0b[:, 512:1024])
        nc.vector.tensor_copy(qt[1][:, 0:256], ps_q1a)
        nc.vector.tensor_copy(qt[1][:, 256:512], ps_q1b)

        # ---------------- fillers ----------------
        MM = 0.427  # us per 512-col matmul at full clock (cost bookkeeping)

        def q1cd_gen():
            # q1 quarters C/D through the freed attnv slots (xt resident)
            ps_c = ps_av.tile([P, 256], F32, name="ps_q1c", tag="av")
            for ci in range(8):
                mm(ps_c, wqk1_sb[:, ci, 0:P], xt[ci][:, 512:768],
                   start=(ci == 0), stop=(ci == 7), skip_group_check=True)
                yield 107
            ps_d = ps_av.tile([P, 256], F32, name="ps_q1d", tag="av")
            for ci in range(8):
                mm(ps_d, wqk1_sb[:, ci, 0:P], xt[ci][:, 768:1024],
                   start=(ci == 0), stop=(ci == 7), skip_group_check=True)
                yield 107
            nc.vector.tensor_copy(qt[1][:, 512:768], ps_c)
            nc.vector.tensor_copy(qt[1][:, 768:1024], ps_d)
            yield 0

        def k1_gen(quarter):
            ps = ps_sq.tile([P, 512], F32, name=f"ps_k1{quarter}", tag="sq")
            nk0 = quarter * 512
            for ci in range(8):
                lw = wqk1_sb[:, ci, P:2 * P]
                mm(ps, lw, xt[ci][:, nk0:nk0 + 512],
                   start=(ci == 0), stop=(ci == 7), skip_group_check=True)
                yield 213
            nc.vector.tensor_copy(kt[1][:, nk0:nk0 + 512], ps)
            yield 0

        def v_gen(t):
            # 2 kv-blocks (j = 2t, 2t+1) share one psum slot; one wide evac
            ps = ps_sq.tile([P, 2, DH], F32, name=f"ps_v{t}", tag="sq")
            for ci in range(8):
                for jj in range(2):
                    j = 2 * t + jj
                    # one start per psum BANK: jj=1's first matmul relies on
                    # the pending-zero left by jj=0's start
                    mm(ps[:, jj, :], xt[ci][:, j * P:(j + 1) * P],
                       wv_sb[:, ci, :],
                       start=(ci == 0 and jj == 0),
                       stop=(ci == 7 and jj == 1), skip_group_check=True)
                yield 214
            nc.vector.tensor_copy(
                v_sb[:, 2 * t:2 * t + 2, :, 0:D],
                ps.rearrange("p j (h d) -> p j h d", h=HPC))
            yield 0

        def proj0_gen(m):
            # two independent half-column units -> 2-wide through the sq slots
            for nh in range(2):
                ps = ps_sq.tile([P, 512], F32, name=f"ps_pj0_{m}_{nh}", tag="sq")
                mm(ps, attn_T[0][:, m, :],
                   wp_sb[0][:, nh * 512:(nh + 1) * 512],
                   start=True, stop=True, skip_group_check=True)
                yield 213
                fin = finp.tile([P, 512], BF16, name=f"fin0_{m}_{nh}", tag="fin")
                nc.vector.tensor_copy(fin, ps)
                nc.sync.dma_start(
                    out=outA[m * P:(m + 1) * P, nh * 512:(nh + 1) * 512],
                    in_=fin)
                yield 0

        # ---------------- attention pieces ----------------
        av_tiles = {}

        def alloc_av(h):
            av_tiles[h] = [ps_av.tile([P, 4, D + 1], F32, name=f"av{h}_{s}",
                                      tag="av") for s in range(2)]

        ets = {}

        def scores_j(h, j):
            pair, po = h // 2, D * (h % 2)
            ps = ps_sc.tile([P, NQ], F32, name=f"ps_s{h}_{j}", tag="sc")
            lw = kt[pair][po:po + D, j * P:(j + 1) * P]
            for nh in range(2):
                mm(ps[:, nh * 512:(nh + 1) * 512], lw,
                   qt[pair][po:po + D, nh * 512:(nh + 1) * 512],
                   start=True, stop=True, skip_group_check=True)
            et = ets_pool.tile([P, NQ], BF16, name=f"et{h}_{j}", tag="ets")
            if (h, j) == (3, 15):
                # split the LAST exp so the tail's first attnv half (q-blocks
                # 0..3 read columns 0:512) starts half an exp earlier
                nc.scalar.activation(out=et[:, 0:512], in_=ps[:, 0:512],
                                     func=Exp, scale=SCALE)
                nc.scalar.activation(out=et[:, 512:1024], in_=ps[:, 512:1024],
                                     func=Exp, scale=SCALE)
            else:
                nc.scalar.activation(out=et, in_=ps, func=Exp, scale=SCALE)
            ets[(h, j)] = et

        def attnv_j(h, j):
            et = ets[(h, j)]
            for qb in range(8):
                av = av_tiles[h][qb // 4]
                mm(av[:, qb % 4, :],
                   et[:, qb * P:(qb + 1) * P],
                   v_sb[:, j, h, :],
                   start=(j == 0 and qb % 4 == 0),
                   stop=(j == 15 and qb % 4 == 3),
                   skip_group_check=True)

        def norm_half(h, part, tail):
            # tail=False: DVE + Pool (ACT is mid-exp-stream); tail=True:
            # DVE + ACT (lower latency, stream over)
            pair, half = h // 2, h % 2
            av = av_tiles[h][part]
            nc.vector.reciprocal(rcp[:, h, part * 4:(part + 1) * 4], av[:, :, D])
            for i in range(4):
                qb = part * 4 + i
                dst = attn_pack[pair][:, qb, half * D:(half + 1) * D]
                if tail and i % 2 == 1:
                    nc.scalar.activation(out=dst, in_=av[:, i, 0:D], func=Copy,
                                         scale=rcp[:, h, qb:qb + 1])
                else:
                    nc.vector.tensor_scalar_mul(dst, av[:, i, 0:D],
                                                rcp[:, h, qb:qb + 1])

        def pull(gen, budget):
            # cost-aware: drain up to ~budget ns of emitted matmul work
            acc = 0
            while acc < budget:
                c = next(gen, None)
                if c is None:
                    return False
                acc += c
            return True

        # ---------------- head loops (ACT exp stream is the pacer) --------
        # Each head's 16 exps give ~17.1us of ACT; scores are ~6.8us of PE,
        # leaving ~640ns/iter of PE filler budget.
        # h0: v pairs 0..4 (j0..9) + q1 quarters C/D
        f = chain(v_gen(0), v_gen(1), q1cd_gen(), v_gen(2), v_gen(3), v_gen(4))
        for j in range(16):
            scores_j(0, j)
            pull(f, 640)
        for _ in f:
            pass

        # h1: k1a + k1b first (unblocks h2 scores), then v pairs 5, 6
        f = chain(*(k1_gen(qu) for qu in range(4)), *(v_gen(t) for t in (5, 6)))
        for j in range(16):
            scores_j(1, j)
            pull(f, 615)
        for _ in f:
            pass

        # h2: v pair 7 early + attnv(h0) iters 0..4, norm(h0)@5,
        #     attnv(h1) 6..11, norm(h1)@12, pair0 transpose @12
        A0 = [(0, 1), (1, 4), (4, 7), (7, 10), (10, 13), (13, 16)]
        A1 = [(0, 3), (3, 5), (5, 8), (8, 10), (10, 13), (13, 16)]
        alloc_av(0)
        fv = chain(v_gen(7))
        fp = chain(*(proj0_gen(m) for m in range(8)))

        def tp0(qb):
            tp = ps_av.tile([P, P], BF16, name=f"tp0_{qb}", tag="av")
            nc.tensor.transpose(tp, attn_pack[0][:, qb, :], identity)
            nc.vector.tensor_copy(attn_T[0][:, qb, :], tp)

        for j in range(16):
            scores_j(2, j)
            if j < 6:
                for jj in range(*A0[j]):
                    attnv_j(0, jj)
            elif j == 6:
                norm_half(0, 0, False)
                norm_half(0, 1, False)
                alloc_av(1)
            elif j < 13:
                for jj in range(*A1[j - 7]):
                    attnv_j(1, jj)
            elif j == 13:
                norm_half(1, 0, False)
                norm_half(1, 1, False)
            elif j >= 14:
                for qb in (2 * (j - 14), 2 * (j - 14) + 1):
                    tp0(qb)
            if j < 4:
                pull(fv, 430)

        # h3: attnv(h2) iters 0..7, norm(h2)@8, attnv(h3) j0..14 iters 8..15,
        #     rest of proj0 spread over all iters
        alloc_av(2)
        for j in range(16):
            scores_j(3, j)
            if j < 2:
                tp0(4 + 2 * j)
                tp0(5 + 2 * j)
            if j < 8:
                attnv_j(2, 2 * j)
                attnv_j(2, 2 * j + 1)
            else:
                if j == 8:
                    norm_half(2, 0, False)
                elif j == 9:
                    norm_half(2, 1, False)
                    alloc_av(3)
                if j >= 9:
                    for jj in range((j - 9) * 15 // 7, (j - 8) * 15 // 7):
                        attnv_j(3, jj)
            if 1 <= j <= 6:
                pull(fp, 520)
            elif j >= 10:
                pull(fp, 570)
        for _ in fp:
            pass

        # ---------------- tail ----------------
        attnv_j(3, 15)

        # Per-qb chain: normalize -> PE transpose -> evac -> project -> fin
        # -> DMA, with DVE/ACT/Pool round-robin so no single evac engine
        # serializes the drain.  proj psums 2-deep via the sc tag.
        av3 = av_tiles[3]
        nc.vector.reciprocal(rcp[:, 3, 0:4], av3[0][:, :, D])
        nc.vector.reciprocal(rcp[:, 3, 4:8], av3[1][:, :, D])

        def mul3(qb):
            dst = attn_pack[1][:, qb, D:2 * D]
            src_ = av3[qb // 4][:, qb % 4, 0:D]
            if qb % 2 == 0:
                nc.vector.tensor_scalar_mul(dst, src_, rcp[:, 3, qb:qb + 1])
            else:
                nc.scalar.activation(out=dst, in_=src_, func=Copy,
                                     scale=rcp[:, 3, qb:qb + 1])

        def tp1(qb):
            tp = ps_sq.tile([P, P], BF16, name=f"tp{qb}", tag="sq")
            nc.tensor.transpose(tp, attn_pack[1][:, qb, :], identity)
            if qb % 2 == 0:
                nc.vector.tensor_copy(attn_T[1][:, qb, :], tp)
            else:
                nc.scalar.copy(attn_T[1][:, qb, :], tp)

        # pair1 projection at full-block granularity: both nh matmuls into
        # one 2-bank sc tile, then a SINGLE fin evacuation per m-block on
        # alternating engines (ACT/DVE) -- fewest per-op overheads; the two
        # engines leapfrog so consecutive blocks' evacuations overlap.

        def proj1(m):
            fin = finp.tile([P, C], BF16, name=f"fin1_{m}", tag="fin")
            ps = ps_sc.tile([P, NQ], F32, name=f"pj1_{m}", tag="sc")
            for nh in range(2):
                mm(ps[:, nh * 512:(nh + 1) * 512],
                   attn_T[1][:, m, :],
                   wp_sb[1][:, nh * 512:(nh + 1) * 512],
                   start=True, stop=True, skip_group_check=True)
            if m % 2 == 0:
                nc.scalar.copy(fin, ps)
            else:
                nc.vector.tensor_copy(fin, ps)
            nc.sync.dma_start(out=outB[m * P:(m + 1) * P, :], in_=fin)

        mul3(0)
        tp1(0)
        mul3(1)
        tp1(1)
        for qb in range(2, 8):
            mul3(qb)
            tp1(qb)
            proj1(qb - 2)
        proj1(6)
        proj1(7)


def _get_nc():
    if "nc" not in _CACHE:
        _CACHE["nc"] = _build()
    return _CACHE["nc"]


def kernel(x, wq, wk, wv, w_proj, b_proj):
    x = np.asarray(x, dtype=np.float32)
    wq = np.asarray(wq, dtype=np.float32)
    wk = np.asarray(wk, dtype=np.float32)
    wv = np.asarray(wv, dtype=np.float32)
    w_proj = np.asarray(w_proj, dtype=np.float32)
    b_proj = np.asarray(b_proj, dtype=np.float32)

    nc = _get_nc()
    in_maps = []
    for core in range(8):
        b, g = divmod(core, 4)
        s0 = g * DH
        p0 = slice(s0, s0 + P)            # pair0 rows (heads 4g, 4g+1)
        p1 = slice(s0 + P, s0 + 2 * P)    # pair1 rows
        sl = slice(s0, s0 + DH)
        in_maps.append({
            "xT": np.ascontiguousarray(x[b].T).astype(_BF),
            "wqk0": np.ascontiguousarray(
                np.hstack([wq[p0, :].T, wk[p0, :].T])).astype(_BF),
            "wqk1": np.ascontiguousarray(
                np.hstack([wq[p1, :].T, wk[p1, :].T])).astype(_BF),
            "wvT": np.ascontiguousarray(wv[sl, :].T).astype(_BF),
            "wpT": np.ascontiguousarray(w_proj[:, sl].T).astype(_BF),
        })

    res = run_bass_kernel_spmd(nc, in_maps, core_ids=list(range(8)),
                               trace=bool(int(os.environ.get("KERNEL_TRACE", "0"))))
    _CACHE["last_results"] = res
    acc = [np.zeros((NQ, C), np.float32) for _ in range(2)]
    for core in range(8):
        b = core // 4
        acc[b] += res.results[core]["outA"].astype(np.float32)
        acc[b] += res.results[core]["outB"].astype(np.float32)
    full = np.stack(acc)
    full += b_proj[None, None, :]
    return full.astype(np.float32)


# revision 62
# speedup vs baseline: 1.0123x; 1.0009x over previous
"""Cross-attention kernel for Trainium2, 8-core SPMD (v3: bf16 + transposed attnv).

Problem (all fp32):
  x [2, 2048, 1024]; wq/wk/wv/w_proj [1024, 1024]; b_proj [1024]
  q = x[:, :1024] @ wq.T   (16 heads x 64)
  k, v = x @ wk.T, x @ wv.T
  out = softmax(q k^T / 8) v  -> proj + bias  -> [2, 1024, 1024]

Sharding: 8 cores = 2 (batch) x 4 (head-groups of 4 heads = 2 pairs of 2).
Each core emits TWO bf16 partials (one per head-pair); host upcasts, sums
the 16 partials per batch and adds the bias.

Design (matmul cost = out-cols x 0.4167ns x cpr; bf16 cpr=1 at any width,
fp32r cpr=4 below 256 cols):
  - x/weights stream in as bf16 (half DMA bytes); q/k kept fp32r so scores
    stay high precision; exp output, v, attn, proj all bf16.
  - attnv is transposed: stationary = exp tile [128kv, 128q], moving =
    v [128kv, 65] -> psum [q-block, 65].  8320 cols/head vs 16384, and the
    ones-column denominator lands per-PARTITION, so normalization is a
    cheap DVE tensor_scalar multiply (no PE broadcast matmuls).
  - normalized attn for a head-pair is packed [128q, 128dd], transposed
    (pair0: DMA-xbar mid-kernel; pair1: PE transpose in the tail where
    PSUM is free) and projected with a full-128 contraction.
  - PSUM (8 banks): scores [128, 1024] x2 (4) + attnv 2 x [128, 4, 65]
    (2) + one [128, 1024] rotating "seq" slot (2) for k1a/k1b/v_j/proj0.
    q pair1 runs inside stage A (its own psum there is the seq slot's
    first user).  Tail projection alternates the sc and seq tags for
    2-deep pipelining.
  - The exp stream (64 x [128, 1024], ~1.07us each) is the pacing engine;
    PE in-loop work is levelled across the 4 head loops so ACT never
    starves: h0 carries v j0..11, h1 carries k1 + v j12..15 + attnv(h0),
    h2 carries attnv(h1), h3 carries attnv(h2) + attnv(h3) + proj0.
"""

import os
import numpy as np
import ml_dtypes

import concourse.bacc as bacc
import concourse.bass as bass
import concourse.tile as tile
import concourse.mybir as mybir
from concourse.bass_utils import run_bass_kernel_spmd
from concourse.masks import make_identity

F32 = mybir.dt.float32
F32R = mybir.dt.float32r
BF16 = mybir.dt.bfloat16

C = 1024          # model dim
N = 2048          # kv tokens
NQ = 1024         # query tokens
HPC = 4           # heads per core
D = 64            # head dim
DH = HPC * D      # per-core slice of C (256)
SCALE = D ** -0.5
P = 128

_CACHE: dict = {}
_BF = ml_dtypes.bfloat16


def _build():
    nc = bacc.Bacc("TRN2", target_bir_lowering=False, debug=False, num_devices=8)

    xT = nc.dram_tensor("xT", [C, N], BF16, kind="ExternalInput").ap()
    # wqk{p} = hstack(wq[pair p slice].T, wk[pair p slice].T)  [C, 256]
    wqk0 = nc.dram_tensor("wqk0", [C, 2 * P], BF16, kind="ExternalInput").ap()
    wqk1 = nc.dram_tensor("wqk1", [C, 2 * P], BF16, kind="ExternalInput").ap()
    wvT = nc.dram_tensor("wvT", [C, DH], BF16, kind="ExternalInput").ap()
    wpT = nc.dram_tensor("wpT", [DH, C], BF16, kind="ExternalInput").ap()
    outA = nc.dram_tensor("outA", [NQ, C], BF16, kind="ExternalOutput").ap()
    outB = nc.dram_tensor("outB", [NQ, C], BF16, kind="ExternalOutput").ap()

    with tile.TileContext(nc) as tc, \
            nc.allow_low_precision(reason="bf16 pipeline within 2e-2 tolerance"):
        _emit(tc, xT, wqk0, wqk1, wvT, wpT, outA, outB)

    nc.compile()
    return nc


def _emit(tc, xT, wqk0, wqk1, wvT, wpT, outA, outB):
    nc = tc.nc
    mm = nc.tensor.matmul
    Exp = mybir.ActivationFunctionType.Exp
    Copy = mybir.ActivationFunctionType.Copy

    from contextlib import ExitStack
    from itertools import chain

    with ExitStack() as ctx:
        singles = ctx.enter_context(tc.tile_pool(name="singles", bufs=1))
        ets_pool = ctx.enter_context(tc.tile_pool(name="ets", bufs=32))
        finp = ctx.enter_context(tc.tile_pool(name="finp", bufs=8))
        ps_sc = ctx.enter_context(tc.tile_pool(name="ps_sc", bufs=2, space="PSUM"))
        ps_av = ctx.enter_context(tc.tile_pool(name="ps_av", bufs=2, space="PSUM"))
        ps_sq = ctx.enter_context(tc.tile_pool(name="ps_sq", bufs=2, space="PSUM"))

        # ---------------- input DMAs (one ordered SP/HWDGE stream) --------
        # Per chunk: wqk0_ci, wqk1_ci, x_ci  (stage A consumes q0/k0/q1 per
        # chunk as it lands); then wv, wp (needed from ~h0/h3).
        xt = [singles.tile([P, N], BF16, name=f"xt{ci}", tag=f"xt{ci}")
              for ci in range(8)]
        wqk0_sb = singles.tile([P, 8, 2 * P], BF16, name="wqk0_sb", tag="wqk0")
        wqk1_sb = singles.tile([P, 8, 2 * P], BF16, name="wqk1_sb", tag="wqk1")
        wv_sb = singles.tile([P, 8, DH], BF16, name="wv_sb", tag="wv")
        wp_sb = [singles.tile([P, C], BF16, name=f"wp{p}", tag=f"wp{p}")
                 for p in range(2)]

        wqk0_src = wqk0.rearrange("(a p) d -> p a d", p=P)
        wqk1_src = wqk1.rearrange("(a p) d -> p a d", p=P)
        wv_src = wvT.rearrange("(a p) d -> p a d", p=P)

        for ci in range(8):
            nc.sync.dma_start(out=wqk0_sb[:, ci, :], in_=wqk0_src[:, ci, :])
            if ci >= 6:
                nc.sync.dma_start(out=xt[ci][:, 0:1024],
                                  in_=xT[ci * P:(ci + 1) * P, 0:1024])
                nc.sync.dma_start(out=xt[ci][:, 1024:2048],
                                  in_=xT[ci * P:(ci + 1) * P, 1024:2048])
            else:
                nc.sync.dma_start(out=xt[ci], in_=xT[ci * P:(ci + 1) * P, :])
        for ci in range(8):
            nc.sync.dma_start(out=wqk1_sb[:, ci, :], in_=wqk1_src[:, ci, :])
            nc.sync.dma_start(out=wv_sb[:, ci, :], in_=wv_src[:, ci, :])
        for p in range(2):
            nc.sync.dma_start(out=wp_sb[p], in_=wpT[p * P:(p + 1) * P, :])

        # ---------------- small consts ----------------
        identity = singles.tile([P, P], BF16, name="identity", tag="ident")
        make_identity(nc, identity)

        # Pre-trigger the exp table load while DMAs stream.
        dmt = singles.tile([1, 1], BF16, name="dmt", tag="dmt")
        nc.scalar.activation(out=dmt, in_=identity[0:1, 0:1], func=Exp, scale=1.0)

        # ---------------- persistent SBUF ----------------
        qt = [singles.tile([P, NQ], F32R, name=f"qt{p}", tag=f"qt{p}")
              for p in range(2)]
        kt = [singles.tile([P, N], F32R, name=f"kt{p}", tag=f"kt{p}")
              for p in range(2)]
        v_sb = singles.tile([P, 16, HPC, D + 1], BF16, name="v_sb", tag="v_sb")
        nc.gpsimd.memset(v_sb[:, :, :, D:D + 1], 1.0)

        attn_pack = [singles.tile([P, 8, P], BF16, name=f"apk{p}", tag=f"apk{p}")
                     for p in range(2)]
        attn_T = [singles.tile([P, 8, P], BF16, name=f"atT{p}", tag=f"atT{p}")
                  for p in range(2)]
        rcp = singles.tile([P, HPC, 8], F32, name="rcp", tag="rcp")

        # ---------------- stage A: q pair0 + k pair0 + q1 quarters --------
        # 8 mm per chunk vs ~1.6us chunk arrival: roughly DMA-paced.  q1 is
        # split into four [128, 256] quarter-psums so two of them fit the
        # (otherwise idle) 1-bank attnv slots during stage A; the other two
        # run as the first h0 fillers.
        ps_q0a = ps_sq.tile([P, 512], F32, name="ps_q0a", tag="sq")
        ps_q0b = ps_sq.tile([P, 512], F32, name="ps_q0b", tag="sq")
        ps_k0a = ps_sc.tile([P, NQ], F32, name="ps_k0a", tag="sc")
        ps_k0b = ps_sc.tile([P, NQ], F32, name="ps_k0b", tag="sc")
        ps_q1a = ps_av.tile([P, 256], F32, name="ps_q1a", tag="av")
        ps_q1b = ps_av.tile([P, 256], F32, name="ps_q1b", tag="av")
        for ci in range(8):
            lw_q0 = wqk0_sb[:, ci, 0:P]
            lw_k0 = wqk0_sb[:, ci, P:2 * P]
            lw_q1 = wqk1_sb[:, ci, 0:P]
            st = dict(start=(ci == 0), stop=(ci == 7), skip_group_check=True)
            mm(ps_q0a, lw_q0, xt[ci][:, 0:512], **st)
            mm(ps_k0a[:, 0:512], lw_k0, xt[ci][:, 0:512], **st)
            mm(ps_q0b, lw_q0, xt[ci][:, 512:1024], **st)
            mm(ps_k0a[:, 512:1024], lw_k0, xt[ci][:, 512:1024], **st)
            mm(ps_k0b[:, 0:512], lw_k0, xt[ci][:, 1024:1536], **st)
            mm(ps_k0b[:, 512:1024], lw_k0, xt[ci][:, 1536:2048], **st)
            mm(ps_q1a, lw_q1, xt[ci][:, 0:256], **st)
            mm(ps_q1b, lw_q1, xt[ci][:, 256:512], **st)
        # evacs split across DVE+ACT+Pool (all idle pre-stream) to shorten
        # the serial path to the first scores matmul
        nc.vector.tensor_copy(qt[0][:, 0:512], ps_q0a)
        nc.scalar.copy(qt[0][:, 512:1024], ps_q0b)
        nc.vector.tensor_copy(kt[0][:, 0:512], ps_k0a[:, 0:512])
        nc.scalar.copy(kt[0][:, 512:1024], ps_k0a[:, 512:1024])
        nc.vector.tensor_copy(kt[0][:, 1024:1536], ps_k0b[:, 0:512])
        nc.scalar.copy(kt[0][:, 1536:2048], ps_k0b[:, 512:1024])
        nc.vector.tensor_copy(qt[1][:, 0:256], ps_q1a)
        nc.vector.tensor_copy(qt[1][:, 256:512], ps_q1b)

        # ---------------- fillers ----------------
        MM = 0.427  # us per 512-col matmul at full clock (cost bookkeeping)

        def q1cd_gen():
            # q1 quarters C/D through the freed attnv slots (xt resident)
            ps_c = ps_av.tile([P, 256], F32, name="ps_q1c", tag="av")
            for ci in range(8):
                mm(ps_c, wqk1_sb[:, ci, 0:P], xt[ci][:, 512:768],
                   start=(ci == 0), stop=(ci == 7), skip_group_check=True)
                yield 107
            ps_d = ps_av.tile([P, 256], F32, name="ps_q1d", tag="av")
            for ci in range(8):
                mm(ps_d, wqk1_sb[:, ci, 0:P], xt[ci][:, 768:1024],
                   start=(ci == 0), stop=(ci == 7), skip_group_check=True)
                yield 107
            nc.vector.tensor_copy(qt[1][:, 512:768], ps_c)
            nc.vector.tensor_copy(qt[1][:, 768:1024], ps_d)
            yield 0

        def k1_gen(quarter):
            ps = ps_sq.tile([P, 512], F32, name=f"ps_k1{quarter}", tag="sq")
            nk0 = quarter * 512
            for ci in range(8):
                lw = wqk1_sb[:, ci, P:2 * P]
                mm(ps, lw, xt[ci][:, nk0:nk0 + 512],
                   start=(ci == 0), stop=(ci == 7), skip_group_check=True)
                yield 213
            nc.vector.tensor_copy(kt[1][:, nk0:nk0 + 512], ps)
            yield 0

        def v_gen(t):
            # 2 kv-blocks (j = 2t, 2t+1) share one psum slot; one wide evac
            ps = ps_sq.tile([P, 2, DH], F32, name=f"ps_v{t}", tag="sq")
            for ci in range(8):
                for jj in range(2):
                    j = 2 * t + jj
                    # one start per psum BANK: jj=1's first matmul relies on
                    # the pending-zero left by jj=0's start
                    mm(ps[:, jj, :], xt[ci][:, j * P:(j + 1) * P],
                       wv_sb[:, ci, :],
                       start=(ci == 0 and jj == 0),
                       stop=(ci == 7 and jj == 1), skip_group_check=True)
                yield 214
            nc.vector.tensor_copy(
                v_sb[:, 2 * t:2 * t + 2, :, 0:D],
                ps.rearrange("p j (h d) -> p j h d", h=HPC))
            yield 0

        def proj0_gen(m):
            # two independent half-column units -> 2-wide through the sq slots
            for nh in range(2):
                ps = ps_sq.tile([P, 512], F32, name=f"ps_pj0_{m}_{nh}", tag="sq")
                mm(ps, attn_T[0][:, m, :],
                   wp_sb[0][:, nh * 512:(nh + 1) * 512],
                   start=True, stop=True, skip_group_check=True)
                yield 213
                fin = finp.tile([P, 512], BF16, name=f"fin0_{m}_{nh}", tag="fin")
                nc.vector.tensor_copy(fin, ps)
                nc.sync.dma_start(
                    out=outA[m * P:(m + 1) * P, nh * 512:(nh + 1) * 512],
                    in_=fin)
                yield 0

        # ---------------- attention pieces ----------------
        av_tiles = {}

        def alloc_av(h):
            av_tiles[h] = [ps_av.tile([P, 4, D + 1], F32, name=f"av{h}_{s}",
                                      tag="av") for s in range(2)]

        ets = {}

        def scores_j(h, j):
            pair, po = h // 2, D * (h % 2)
            ps = ps_sc.tile([P, NQ], F32, name=f"ps_s{h}_{j}", tag="sc")
            lw = kt[pair][po:po + D, j * P:(j + 1) * P]
            for nh in range(2):
                mm(ps[:, nh * 512:(nh + 1) * 512], lw,
                   qt[pair][po:po + D, nh * 512:(nh + 1) * 512],
                   start=True, stop=True, skip_group_check=True)
            et = ets_pool.tile([P, NQ], BF16, name=f"et{h}_{j}", tag="ets")
            if (h, j) == (3, 15):
                # split the LAST exp so the tail's first attnv half (q-blocks
                # 0..3 read columns 0:512) starts half an exp earlier
                nc.scalar.activation(out=et[:, 0:512], in_=ps[:, 0:512],
                                     func=Exp, scale=SCALE)
                nc.scalar.activation(out=et[:, 512:1024], in_=ps[:, 512:1024],
                                     func=Exp, scale=SCALE)
            else:
                nc.scalar.activation(out=et, in_=ps, func=Exp, scale=SCALE)
            ets[(h, j)] = et

        def attnv_j(h, j):
            et = ets[(h, j)]
            for qb in range(8):
                av = av_tiles[h][qb // 4]
                mm(av[:, qb % 4, :],
                   et[:, qb * P:(qb + 1) * P],
                   v_sb[:, j, h, :],
                   start=(j == 0 and qb % 4 == 0),
                   stop=(j == 15 and qb % 4 == 3),
                   skip_group_check=True)

        def norm_half(h, part, tail):
            # tail=False: DVE + Pool (ACT is mid-exp-stream); tail=True:
            # DVE + ACT (lower latency, stream over)
            pair, half = h // 2, h % 2
            av = av_tiles[h][part]
            nc.vector.reciprocal(rcp[:, h, part * 4:(part + 1) * 4], av[:, :, D])
            for i in range(4):
                qb = part * 4 + i
                dst = attn_pack[pair][:, qb, half * D:(half + 1) * D]
                if tail and i % 2 == 1:
                    nc.scalar.activation(out=dst, in_=av[:, i, 0:D], func=Copy,
                                         scale=rcp[:, h, qb:qb + 1])
                else:
                    nc.vector.tensor_scalar_mul(dst, av[:, i, 0:D],
                                                rcp[:, h, qb:qb + 1])

        def pull(gen, budget):
            # cost-aware: drain up to ~budget ns of emitted matmul work
            acc = 0
            while acc < budget:
                c = next(gen, None)
                if c is None:
                    return False
                acc += c
            return True

        # ---------------- head loops (ACT exp stream is the pacer) --------
        # Each head's 16 exps give ~17.1us of ACT; scores are ~6.8us of PE,
        # leaving ~640ns/iter of PE filler budget.
        # h0: v pairs 0..4 (j0..9) + q1 quarters C/D
        f = chain(v_gen(0), v_gen(1), q1cd_gen(), v_gen(2), v_gen(3), v_gen(4))
        for j in range(16):
            scores_j(0, j)
            pull(f, 640)
        for _ in f:
            pass

        # h1: k1a + k1b first (unblocks h2 scores), then v pairs 5, 6
        f = chain(*(k1_gen(qu) for qu in range(4)), *(v_gen(t) for t in (5, 6)))
        for j in range(16):
            scores_j(1, j)
            pull(f, 615)
        for _ in f:
            pass

        # h2: v pair 7 early + attnv(h0) iters 0..4, norm(h0)@5,
        #     attnv(h1) 6..11, norm(h1)@12, pair0 transpose @12
        A0 = [(0, 1), (1, 4), (4, 7), (7, 10), (10, 13), (13, 16)]
        A1 = [(0, 3), (3, 5), (5, 8), (8, 10), (10, 13), (13, 16)]
        alloc_av(0)
        fv = chain(v_gen(7))
        fp = chain(*(proj0_gen(m) for m in range(8)))

        def tp0(qb):
            tp = ps_av.tile([P, P], BF16, name=f"tp0_{qb}", tag="av")
            nc.tensor.transpose(tp, attn_pack[0][:, qb, :], identity)
            nc.vector.tensor_copy(attn_T[0][:, qb, :], tp)

        for j in range(16):
            scores_j(2, j)
            if j < 6:
                for jj in range(*A0[j]):
                    attnv_j(0, jj)
            elif j == 6:
                norm_half(0, 0, False)
                norm_half(0, 1, False)
                alloc_av(1)
            elif j < 13:
                for jj in range(*A1[j - 7]):
                    attnv_j(1, jj)
            elif j == 13:
                norm_half(1, 0, False)
                norm_half(1, 1, False)
            elif j >= 14:
                for qb in (2 * (j - 14), 2 * (j - 14) + 1):
                    tp0(qb)
            if j < 4:
                pull(fv, 430)

        # h3: attnv(h2) iters 0..7, norm(h2)@8, attnv(h3) j0..14 iters 8..15,
        #     rest of proj0 spread over all iters
        alloc_av(2)
        for j in range(16):
            scores_j(3, j)
            if j < 2:
                tp0(4 + 2 * j)
                tp0(5 + 2 * j)
            if j < 8:
                attnv_j(2, 2 * j)
                attnv_j(2, 2 * j + 1)
            else:
                if j == 8:
                    norm_half(2, 0, False)
                elif j == 9:
                    norm_half(2, 1, False)
                    alloc_av(3)
                if j >= 9:
                    for jj in range((j - 9) * 15 // 7, (j - 8) * 15 // 7):
                        attnv_j(3, jj)
            if 1 <= j <= 6:
                pull(fp, 520)
            elif j >= 10:
                pull(fp, 570)
        for _ in fp:
            pass

        # ---------------- tail ----------------
        attnv_j(3, 15)

        # Per-qb chain: normalize -> PE transpose -> evac -> project -> fin
        # -> DMA, with DVE/ACT/Pool round-robin so no single evac engine
        # serializes the drain.  proj psums 2-deep via the sc tag.
        av3 = av_tiles[3]
        nc.vector.reciprocal(rcp[:, 3, 0:4], av3[0][:, :, D])
        nc.vector.reciprocal(rcp[:, 3, 4:8], av3[1][:, :, D])

        def mul3(qb):
            dst = attn_pack[1][:, qb, D:2 * D]
            src_ = av3[qb // 4][:, qb % 4, 0:D]
            nc.vector.tensor_scalar_mul(dst, src_, rcp[:, 3, qb:qb + 1])

        def tp1(qb):
            tp = ps_sq.tile([P, P], BF16, name=f"tp{qb}", tag="sq")
            nc.tensor.transpose(tp, attn_pack[1][:, qb, :], identity)
            if qb % 2 == 0:
                nc.vector.tensor_copy(attn_T[1][:, qb, :], tp)
            else:
                nc.scalar.copy(attn_T[1][:, qb, :], tp)

        # pair1 projection at full-block granularity: both nh matmuls into
        # one 2-bank sc tile, then a SINGLE fin evacuation per m-block on
        # alternating engines (ACT/DVE) -- fewest per-op overheads; the two
        # engines leapfrog so consecutive blocks' evacuations overlap.

        def proj1(m):
            fin = finp.tile([P, C], BF16, name=f"fin1_{m}", tag="fin")
            ps = ps_sc.tile([P, NQ], F32, name=f"pj1_{m}", tag="sc")
            for nh in range(2):
                mm(ps[:, nh * 512:(nh + 1) * 512],
                   attn_T[1][:, m, :],
                   wp_sb[1][:, nh * 512:(nh + 1) * 512],
                   start=True, stop=True, skip_group_check=True)
            if m % 2 == 0:
                nc.scalar.copy(fin, ps)
            else:
                nc.vector.tensor_copy(fin, ps)
            nc.sync.dma_start(out=outB[m * P:(m + 1) * P, :], in_=fin)

        mul3(0)
        tp1(0)
        mul3(1)
        tp1(1)
        for qb in range(2, 8):
            mul3(qb)
            tp1(qb)
            proj1(qb - 2)
        proj1(6)
        proj1(7)


def _get_nc():
    if "nc" not in _CACHE:
        _CACHE["nc"] = _build()
    return _CACHE["nc"]


def kernel(x, wq, wk, wv, w_proj, b_proj):
    x = np.asarray(x, dtype=np.float32)
    wq = np.asarray(wq, dtype=np.float32)
    wk = np.asarray(wk, dtype=np.float32)
    wv = np.asarray(wv, dtype=np.float32)
    w_proj = np.asarray(w_proj, dtype=np.float32)
    b_proj = np.asarray(b_proj, dtype=np.float32)

    nc = _get_nc()
    in_maps = []
    for core in range(8):
        b, g = divmod(core, 4)
        s0 = g * DH
        p0 = slice(s0, s0 + P)            # pair0 rows (heads 4g, 4g+1)
        p1 = slice(s0 + P, s0 + 2 * P)    # pair1 rows
        sl = slice(s0, s0 + DH)
        in_maps.append({
            "xT": np.ascontiguousarray(x[b].T).astype(_BF),
            "wqk0": np.ascontiguousarray(
                np.hstack([wq[p0, :].T, wk[p0, :].T])).astype(_BF),
            "wqk1": np.ascontiguousarray(
                np.hstack([wq[p1, :].T, wk[p1, :].T])).astype(_BF),
            "wvT": np.ascontiguousarray(wv[sl, :].T).astype(_BF),
            "wpT": np.ascontiguousarray(w_proj[:, sl].T).astype(_BF),
        })

    res = run_bass_kernel_spmd(nc, in_maps, core_ids=list(range(8)),
                               trace=bool(int(os.environ.get("KERNEL_TRACE", "0"))))
    _CACHE["last_results"] = res
    acc = [np.zeros((NQ, C), np.float32) for _ in range(2)]
    for core in range(8):
        b = core // 4
        acc[b] += res.results[core]["outA"].astype(np.float32)
        acc[b] += res.results[core]["outB"].astype(np.float32)
    full = np.stack(acc)
    full += b_proj[None, None, :]
    return full.astype(np.float32)


# revision 65
# speedup vs baseline: 1.0136x; 1.0013x over previous
"""Cross-attention kernel for Trainium2, 8-core SPMD (v3: bf16 + transposed attnv).

Problem (all fp32):
  x [2, 2048, 1024]; wq/wk/wv/w_proj [1024, 1024]; b_proj [1024]
  q = x[:, :1024] @ wq.T   (16 heads x 64)
  k, v = x @ wk.T, x @ wv.T
  out = softmax(q k^T / 8) v  -> proj + bias  -> [2, 1024, 1024]

Sharding: 8 cores = 2 (batch) x 4 (head-groups of 4 heads = 2 pairs of 2).
Each core emits TWO bf16 partials (one per head-pair); host upcasts, sums
the 16 partials per batch and adds the bias.

Design (matmul cost = out-cols x 0.4167ns x cpr; bf16 cpr=1 at any width,
fp32r cpr=4 below 256 cols):
  - x/weights stream in as bf16 (half DMA bytes); q/k kept fp32r so scores
    stay high precision; exp output, v, attn, proj all bf16.
  - attnv is transposed: stationary = exp tile [128kv, 128q], moving =
    v [128kv, 65] -> psum [q-block, 65].  8320 cols/head vs 16384, and the
    ones-column denominator lands per-PARTITION, so normalization is a
    cheap DVE tensor_scalar multiply (no PE broadcast matmuls).
  - normalized attn for a head-pair is packed [128q, 128dd], transposed
    (pair0: DMA-xbar mid-kernel; pair1: PE transpose in the tail where
    PSUM is free) and projected with a full-128 contraction.
  - PSUM (8 banks): scores [128, 1024] x2 (4) + attnv 2 x [128, 4, 65]
    (2) + one [128, 1024] rotating "seq" slot (2) for k1a/k1b/v_j/proj0.
    q pair1 runs inside stage A (its own psum there is the seq slot's
    first user).  Tail projection alternates the sc and seq tags for
    2-deep pipelining.
  - The exp stream (64 x [128, 1024], ~1.07us each) is the pacing engine;
    PE in-loop work is levelled across the 4 head loops so ACT never
    starves: h0 carries v j0..11, h1 carries k1 + v j12..15 + attnv(h0),
    h2 carries attnv(h1), h3 carries attnv(h2) + attnv(h3) + proj0.
"""

import os
import numpy as np
import ml_dtypes

import concourse.bacc as bacc
import concourse.bass as bass
import concourse.tile as tile
import concourse.mybir as mybir
from concourse.bass_utils import run_bass_kernel_spmd
from concourse.masks import make_identity

F32 = mybir.dt.float32
F32R = mybir.dt.float32r
BF16 = mybir.dt.bfloat16

C = 1024          # model dim
N = 2048          # kv tokens
NQ = 1024         # query tokens
HPC = 4           # heads per core
D = 64            # head dim
DH = HPC * D      # per-core slice of C (256)
SCALE = D ** -0.5
P = 128

_CACHE: dict = {}
_BF = ml_dtypes.bfloat16


def _build():
    nc = bacc.Bacc("TRN2", target_bir_lowering=False, debug=False, num_devices=8)

    xT = nc.dram_tensor("xT", [C, N], BF16, kind="ExternalInput").ap()
    # wqk{p} = hstack(wq[pair p slice].T, wk[pair p slice].T)  [C, 256]
    wqk0 = nc.dram_tensor("wqk0", [C, 2 * P], BF16, kind="ExternalInput").ap()
    wqk1 = nc.dram_tensor("wqk1", [C, 2 * P], BF16, kind="ExternalInput").ap()
    wvT = nc.dram_tensor("wvT", [C, DH], BF16, kind="ExternalInput").ap()
    wpT = nc.dram_tensor("wpT", [DH, C], BF16, kind="ExternalInput").ap()
    outA = nc.dram_tensor("outA", [NQ, C], BF16, kind="ExternalOutput").ap()
    outB = nc.dram_tensor("outB", [NQ, C], BF16, kind="ExternalOutput").ap()

    with tile.TileContext(nc) as tc, \
            nc.allow_low_precision(reason="bf16 pipeline within 2e-2 tolerance"):
        _emit(tc, xT, wqk0, wqk1, wvT, wpT, outA, outB)

    nc.compile()
    return nc


def _emit(tc, xT, wqk0, wqk1, wvT, wpT, outA, outB):
    nc = tc.nc
    mm = nc.tensor.matmul
    Exp = mybir.ActivationFunctionType.Exp
    Copy = mybir.ActivationFunctionType.Copy

    from contextlib import ExitStack
    from itertools import chain

    with ExitStack() as ctx:
        singles = ctx.enter_context(tc.tile_pool(name="singles", bufs=1))
        ets_pool = ctx.enter_context(tc.tile_pool(name="ets", bufs=32))
        finp = ctx.enter_context(tc.tile_pool(name="finp", bufs=8))
        ps_sc = ctx.enter_context(tc.tile_pool(name="ps_sc", bufs=2, space="PSUM"))
        ps_av = ctx.enter_context(tc.tile_pool(name="ps_av", bufs=2, space="PSUM"))
        ps_sq = ctx.enter_context(tc.tile_pool(name="ps_sq", bufs=2, space="PSUM"))

        # ---------------- input DMAs (one ordered SP/HWDGE stream) --------
        # Per chunk: wqk0_ci, wqk1_ci, x_ci  (stage A consumes q0/k0/q1 per
        # chunk as it lands); then wv, wp (needed from ~h0/h3).
        xt = [singles.tile([P, N], BF16, name=f"xt{ci}", tag=f"xt{ci}")
              for ci in range(8)]
        wqk0_sb = singles.tile([P, 8, 2 * P], BF16, name="wqk0_sb", tag="wqk0")
        wqk1_sb = singles.tile([P, 8, 2 * P], BF16, name="wqk1_sb", tag="wqk1")
        wv_sb = singles.tile([P, 8, DH], BF16, name="wv_sb", tag="wv")
        wp_sb = [singles.tile([P, C], BF16, name=f"wp{p}", tag=f"wp{p}")
                 for p in range(2)]

        wqk0_src = wqk0.rearrange("(a p) d -> p a d", p=P)
        wqk1_src = wqk1.rearrange("(a p) d -> p a d", p=P)
        wv_src = wvT.rearrange("(a p) d -> p a d", p=P)

        for ci in range(8):
            nc.sync.dma_start(out=wqk0_sb[:, ci, :], in_=wqk0_src[:, ci, :])
            if ci >= 6:
                nc.sync.dma_start(out=xt[ci][:, 0:1024],
                                  in_=xT[ci * P:(ci + 1) * P, 0:1024])
                nc.sync.dma_start(out=xt[ci][:, 1024:2048],
                                  in_=xT[ci * P:(ci + 1) * P, 1024:2048])
            else:
                nc.sync.dma_start(out=xt[ci], in_=xT[ci * P:(ci + 1) * P, :])
        for ci in range(8):
            nc.sync.dma_start(out=wqk1_sb[:, ci, :], in_=wqk1_src[:, ci, :])
            nc.sync.dma_start(out=wv_sb[:, ci, :], in_=wv_src[:, ci, :])
        for p in range(2):
            nc.sync.dma_start(out=wp_sb[p], in_=wpT[p * P:(p + 1) * P, :])

        # ---------------- small consts ----------------
        identity = singles.tile([P, P], BF16, name="identity", tag="ident")
        make_identity(nc, identity)

        # Pre-trigger the exp table load while DMAs stream.
        dmt = singles.tile([1, 1], BF16, name="dmt", tag="dmt")
        nc.scalar.activation(out=dmt, in_=identity[0:1, 0:1], func=Exp, scale=1.0)

        # ---------------- persistent SBUF ----------------
        qt = [singles.tile([P, NQ], F32R, name=f"qt{p}", tag=f"qt{p}")
              for p in range(2)]
        kt = [singles.tile([P, N], F32R, name=f"kt{p}", tag=f"kt{p}")
              for p in range(2)]
        v_sb = singles.tile([P, 16, HPC, D + 1], BF16, name="v_sb", tag="v_sb")
        nc.gpsimd.memset(v_sb[:, :, :, D:D + 1], 1.0)

        attn_pack = [singles.tile([P, 8, P], BF16, name=f"apk{p}", tag=f"apk{p}")
                     for p in range(2)]
        attn_T = [singles.tile([P, 8, P], BF16, name=f"atT{p}", tag=f"atT{p}")
                  for p in range(2)]
        rcp = singles.tile([P, HPC, 8], F32, name="rcp", tag="rcp")

        # ---------------- stage A: q pair0 + k pair0 + q1 quarters --------
        # 8 mm per chunk vs ~1.6us chunk arrival: roughly DMA-paced.  q1 is
        # split into four [128, 256] quarter-psums so two of them fit the
        # (otherwise idle) 1-bank attnv slots during stage A; the other two
        # run as the first h0 fillers.
        ps_q0a = ps_sq.tile([P, 512], F32, name="ps_q0a", tag="sq")
        ps_q0b = ps_sq.tile([P, 512], F32, name="ps_q0b", tag="sq")
        ps_k0a = ps_sc.tile([P, NQ], F32, name="ps_k0a", tag="sc")
        ps_k0b = ps_sc.tile([P, NQ], F32, name="ps_k0b", tag="sc")
        ps_q1a = ps_av.tile([P, 256], F32, name="ps_q1a", tag="av")
        ps_q1b = ps_av.tile([P, 256], F32, name="ps_q1b", tag="av")
        for ci in range(8):
            lw_q0 = wqk0_sb[:, ci, 0:P]
            lw_k0 = wqk0_sb[:, ci, P:2 * P]
            lw_q1 = wqk1_sb[:, ci, 0:P]
            st = dict(start=(ci == 0), stop=(ci == 7), skip_group_check=True)
            mm(ps_q0a, lw_q0, xt[ci][:, 0:512], **st)
            mm(ps_k0a[:, 0:512], lw_k0, xt[ci][:, 0:512], **st)
            mm(ps_q0b, lw_q0, xt[ci][:, 512:1024], **st)
            mm(ps_k0a[:, 512:1024], lw_k0, xt[ci][:, 512:1024], **st)
            mm(ps_k0b[:, 0:512], lw_k0, xt[ci][:, 1024:1536], **st)
            mm(ps_k0b[:, 512:1024], lw_k0, xt[ci][:, 1536:2048], **st)
            mm(ps_q1a, lw_q1, xt[ci][:, 0:256], **st)
            mm(ps_q1b, lw_q1, xt[ci][:, 256:512], **st)
        # evacs split across DVE+ACT+Pool (all idle pre-stream) to shorten
        # the serial path to the first scores matmul
        nc.vector.tensor_copy(qt[0][:, 0:512], ps_q0a)
        nc.scalar.copy(qt[0][:, 512:1024], ps_q0b)
        nc.vector.tensor_copy(kt[0][:, 0:512], ps_k0a[:, 0:512])
        nc.scalar.copy(kt[0][:, 512:1024], ps_k0a[:, 512:1024])
        nc.vector.tensor_copy(kt[0][:, 1024:1536], ps_k0b[:, 0:512])
        nc.scalar.copy(kt[0][:, 1536:2048], ps_k0b[:, 512:1024])
        nc.vector.tensor_copy(qt[1][:, 0:256], ps_q1a)
        nc.vector.tensor_copy(qt[1][:, 256:512], ps_q1b)

        # ---------------- fillers ----------------
        MM = 0.427  # us per 512-col matmul at full clock (cost bookkeeping)

        def q1cd_gen():
            # q1 quarters C/D through the freed attnv slots (xt resident)
            ps_c = ps_av.tile([P, 256], F32, name="ps_q1c", tag="av")
            for ci in range(8):
                mm(ps_c, wqk1_sb[:, ci, 0:P], xt[ci][:, 512:768],
                   start=(ci == 0), stop=(ci == 7), skip_group_check=True)
                yield 107
            ps_d = ps_av.tile([P, 256], F32, name="ps_q1d", tag="av")
            for ci in range(8):
                mm(ps_d, wqk1_sb[:, ci, 0:P], xt[ci][:, 768:1024],
                   start=(ci == 0), stop=(ci == 7), skip_group_check=True)
                yield 107
            nc.vector.tensor_copy(qt[1][:, 512:768], ps_c)
            nc.vector.tensor_copy(qt[1][:, 768:1024], ps_d)
            yield 0

        def k1_gen(quarter):
            ps = ps_sq.tile([P, 512], F32, name=f"ps_k1{quarter}", tag="sq")
            nk0 = quarter * 512
            for ci in range(8):
                lw = wqk1_sb[:, ci, P:2 * P]
                mm(ps, lw, xt[ci][:, nk0:nk0 + 512],
                   start=(ci == 0), stop=(ci == 7), skip_group_check=True)
                yield 213
            nc.vector.tensor_copy(kt[1][:, nk0:nk0 + 512], ps)
            yield 0

        def v_gen(t):
            # 2 kv-blocks (j = 2t, 2t+1) share one psum slot; one wide evac
            ps = ps_sq.tile([P, 2, DH], F32, name=f"ps_v{t}", tag="sq")
            for ci in range(8):
                for jj in range(2):
                    j = 2 * t + jj
                    # one start per psum BANK: jj=1's first matmul relies on
                    # the pending-zero left by jj=0's start
                    mm(ps[:, jj, :], xt[ci][:, j * P:(j + 1) * P],
                       wv_sb[:, ci, :],
                       start=(ci == 0 and jj == 0),
                       stop=(ci == 7 and jj == 1), skip_group_check=True)
                yield 214
            nc.vector.tensor_copy(
                v_sb[:, 2 * t:2 * t + 2, :, 0:D],
                ps.rearrange("p j (h d) -> p j h d", h=HPC))
            yield 0

        def proj0_gen(m):
            # two independent half-column units -> 2-wide through the sq slots
            for nh in range(2):
                ps = ps_sq.tile([P, 512], F32, name=f"ps_pj0_{m}_{nh}", tag="sq")
                mm(ps, attn_T[0][:, m, :],
                   wp_sb[0][:, nh * 512:(nh + 1) * 512],
                   start=True, stop=True, skip_group_check=True)
                yield 213
                fin = finp.tile([P, 512], BF16, name=f"fin0_{m}_{nh}", tag="fin")
                nc.vector.tensor_copy(fin, ps)
                nc.sync.dma_start(
                    out=outA[m * P:(m + 1) * P, nh * 512:(nh + 1) * 512],
                    in_=fin)
                yield 0

        # ---------------- attention pieces ----------------
        av_tiles = {}

        def alloc_av(h):
            av_tiles[h] = [ps_av.tile([P, 4, D + 1], F32, name=f"av{h}_{s}",
                                      tag="av") for s in range(2)]

        ets = {}

        def scores_j(h, j):
            pair, po = h // 2, D * (h % 2)
            ps = ps_sc.tile([P, NQ], F32, name=f"ps_s{h}_{j}", tag="sc")
            lw = kt[pair][po:po + D, j * P:(j + 1) * P]
            for nh in range(2):
                mm(ps[:, nh * 512:(nh + 1) * 512], lw,
                   qt[pair][po:po + D, nh * 512:(nh + 1) * 512],
                   start=True, stop=True, skip_group_check=True)
            et = ets_pool.tile([P, NQ], BF16, name=f"et{h}_{j}", tag="ets")
            if (h, j) == (3, 15):
                # split the LAST exp so the tail's first attnv half (q-blocks
                # 0..3 read columns 0:512) starts half an exp earlier
                nc.scalar.activation(out=et[:, 0:512], in_=ps[:, 0:512],
                                     func=Exp, scale=SCALE)
                nc.scalar.activation(out=et[:, 512:1024], in_=ps[:, 512:1024],
                                     func=Exp, scale=SCALE)
            else:
                nc.scalar.activation(out=et, in_=ps, func=Exp, scale=SCALE)
            ets[(h, j)] = et

        def attnv_j(h, j):
            et = ets[(h, j)]
            for qb in range(8):
                av = av_tiles[h][qb // 4]
                mm(av[:, qb % 4, :],
                   et[:, qb * P:(qb + 1) * P],
                   v_sb[:, j, h, :],
                   start=(j == 0 and qb % 4 == 0),
                   stop=(j == 15 and qb % 4 == 3),
                   skip_group_check=True)

        def norm_half(h, part, tail):
            # tail=False: DVE + Pool (ACT is mid-exp-stream); tail=True:
            # DVE + ACT (lower latency, stream over)
            pair, half = h // 2, h % 2
            av = av_tiles[h][part]
            nc.vector.reciprocal(rcp[:, h, part * 4:(part + 1) * 4], av[:, :, D])
            for i in range(4):
                qb = part * 4 + i
                dst = attn_pack[pair][:, qb, half * D:(half + 1) * D]
                if tail and i % 2 == 1:
                    nc.scalar.activation(out=dst, in_=av[:, i, 0:D], func=Copy,
                                         scale=rcp[:, h, qb:qb + 1])
                else:
                    nc.vector.tensor_scalar_mul(dst, av[:, i, 0:D],
                                                rcp[:, h, qb:qb + 1])

        def pull(gen, budget):
            # cost-aware: drain up to ~budget ns of emitted matmul work
            acc = 0
            while acc < budget:
                c = next(gen, None)
                if c is None:
                    return False
                acc += c
            return True

        # ---------------- head loops (ACT exp stream is the pacer) --------
        # Each head's 16 exps give ~17.1us of ACT; scores are ~6.8us of PE,
        # leaving ~640ns/iter of PE filler budget.
        # h0: v pairs 0..4 (j0..9) + q1 quarters C/D
        f = chain(v_gen(0), v_gen(1), q1cd_gen(), v_gen(2), v_gen(3), v_gen(4))
        for j in range(16):
            scores_j(0, j)
            pull(f, 640)
        for _ in f:
            pass

        # h1: k1a + k1b first (unblocks h2 scores), then v pairs 5, 6
        f = chain(*(k1_gen(qu) for qu in range(4)), *(v_gen(t) for t in (5, 6)))
        for j in range(16):
            scores_j(1, j)
            pull(f, 615)
        for _ in f:
            pass

        # h2: v pair 7 early + attnv(h0) iters 0..4, norm(h0)@5,
        #     attnv(h1) 6..11, norm(h1)@12, pair0 transpose @12
        A0 = [(0, 1), (1, 4), (4, 7), (7, 10), (10, 13), (13, 16)]
        A1 = [(0, 3), (3, 5), (5, 8), (8, 10), (10, 13), (13, 16)]
        alloc_av(0)
        fv = chain(v_gen(7))
        fp = chain(*(proj0_gen(m) for m in range(8)))

        def tp0(qb):
            tp = ps_av.tile([P, P], BF16, name=f"tp0_{qb}", tag="av")
            nc.tensor.transpose(tp, attn_pack[0][:, qb, :], identity)
            nc.vector.tensor_copy(attn_T[0][:, qb, :], tp)

        for j in range(16):
            scores_j(2, j)
            if j < 6:
                for jj in range(*A0[j]):
                    attnv_j(0, jj)
            elif j == 6:
                norm_half(0, 0, False)
                norm_half(0, 1, False)
                alloc_av(1)
            elif j < 13:
                for jj in range(*A1[j - 7]):
                    attnv_j(1, jj)
            elif j == 13:
                norm_half(1, 0, False)
                norm_half(1, 1, False)
            elif j >= 14:
                for qb in (2 * (j - 14), 2 * (j - 14) + 1):
                    tp0(qb)
            if j < 4:
                pull(fv, 430)

        # h3: attnv(h2) iters 0..7, norm(h2)@8, attnv(h3) j0..14 iters 8..15,
        #     rest of proj0 spread over all iters
        alloc_av(2)
        for j in range(16):
            scores_j(3, j)
            if j < 2:
                tp0(4 + 2 * j)
                tp0(5 + 2 * j)
            if j < 8:
                attnv_j(2, 2 * j)
                attnv_j(2, 2 * j + 1)
            else:
                if j == 8:
                    norm_half(2, 0, False)
                elif j == 9:
                    norm_half(2, 1, False)
                    alloc_av(3)
                if j >= 9:
                    for jj in range((j - 9) * 15 // 7, (j - 8) * 15 // 7):
                        attnv_j(3, jj)
            if 1 <= j <= 6:
                pull(fp, 520)
            elif j >= 10:
                pull(fp, 570)
        for _ in fp:
            pass

        # ---------------- tail ----------------
        # last attnv emitted in halves: part0 (q-blocks 0..3, gated only on
        # the first half of the split final exp) releases the rcp/mul/tp/proj
        # chain while the exp's second half still streams
        et15 = ets[(3, 15)]
        for qb in range(4):
            mm(av_tiles[3][qb // 4][:, qb % 4, :],
               et15[:, qb * P:(qb + 1) * P], v_sb[:, 15, 3, :],
               start=False, stop=(qb == 3), skip_group_check=True)

        # Per-qb chain: normalize -> PE transpose -> evac -> project -> fin
        # -> DMA, with DVE/ACT/Pool round-robin so no single evac engine
        # serializes the drain.  proj psums 2-deep via the sc tag.
        av3 = av_tiles[3]
        nc.vector.reciprocal(rcp[:, 3, 0:4], av3[0][:, :, D])

        def mul3(qb):
            dst = attn_pack[1][:, qb, D:2 * D]
            src_ = av3[qb // 4][:, qb % 4, 0:D]
            nc.vector.tensor_scalar_mul(dst, src_, rcp[:, 3, qb:qb + 1])

        def tp1(qb):
            tp = ps_sq.tile([P, P], BF16, name=f"tp{qb}", tag="sq")
            nc.tensor.transpose(tp, attn_pack[1][:, qb, :], identity)
            if qb % 2 == 0:
                nc.vector.tensor_copy(attn_T[1][:, qb, :], tp)
            else:
                nc.scalar.copy(attn_T[1][:, qb, :], tp)

        # pair1 projection at full-block granularity: both nh matmuls into
        # one 2-bank sc tile, then a SINGLE fin evacuation per m-block on
        # alternating engines (ACT/DVE) -- fewest per-op overheads; the two
        # engines leapfrog so consecutive blocks' evacuations overlap.

        def proj1(m):
            fin = finp.tile([P, C], BF16, name=f"fin1_{m}", tag="fin")
            ps = ps_sc.tile([P, NQ], F32, name=f"pj1_{m}", tag="sc")
            for nh in range(2):
                mm(ps[:, nh * 512:(nh + 1) * 512],
                   attn_T[1][:, m, :],
                   wp_sb[1][:, nh * 512:(nh + 1) * 512],
                   start=True, stop=True, skip_group_check=True)
            if m % 2 == 0:
                nc.scalar.copy(fin, ps)
            else:
                nc.vector.tensor_copy(fin, ps)
            nc.sync.dma_start(out=outB[m * P:(m + 1) * P, :], in_=fin)

        mul3(0)
        tp1(0)
        mul3(1)
        tp1(1)
        for qb in range(4, 8):
            mm(av_tiles[3][qb // 4][:, qb % 4, :],
               et15[:, qb * P:(qb + 1) * P], v_sb[:, 15, 3, :],
               start=False, stop=(qb == 7), skip_group_check=True)
        nc.vector.reciprocal(rcp[:, 3, 4:8], av3[1][:, :, D])
        for qb in range(2, 8):
            mul3(qb)
            tp1(qb)
            proj1(qb - 2)
        proj1(6)
        proj1(7)


def _get_nc():
    if "nc" not in _CACHE:
        _CACHE["nc"] = _build()
    return _CACHE["nc"]


def kernel(x, wq, wk, wv, w_proj, b_proj):
    x = np.asarray(x, dtype=np.float32)
    wq = np.asarray(wq, dtype=np.float32)
    wk = np.asarray(wk, dtype=np.float32)
    wv = np.asarray(wv, dtype=np.float32)
    w_proj = np.asarray(w_proj, dtype=np.float32)
    b_proj = np.asarray(b_proj, dtype=np.float32)

    nc = _get_nc()
    in_maps = []
    for core in range(8):
        b, g = divmod(core, 4)
        s0 = g * DH
        p0 = slice(s0, s0 + P)            # pair0 rows (heads 4g, 4g+1)
        p1 = slice(s0 + P, s0 + 2 * P)    # pair1 rows
        sl = slice(s0, s0 + DH)
        in_maps.append({
            "xT": np.ascontiguousarray(x[b].T).astype(_BF),
            "wqk0": np.ascontiguousarray(
                np.hstack([wq[p0, :].T, wk[p0, :].T])).astype(_BF),
            "wqk1": np.ascontiguousarray(
                np.hstack([wq[p1, :].T, wk[p1, :].T])).astype(_BF),
            "wvT": np.ascontiguousarray(wv[sl, :].T).astype(_BF),
            "wpT": np.ascontiguousarray(w_proj[:, sl].T).astype(_BF),
        })

    res = run_bass_kernel_spmd(nc, in_maps, core_ids=list(range(8)),
                               trace=bool(int(os.environ.get("KERNEL_TRACE", "0"))))
    _CACHE["last_results"] = res
    acc = [np.zeros((NQ, C), np.float32) for _ in range(2)]
    for core in range(8):
        b = core // 4
        acc[b] += res.results[core]["outA"].astype(np.float32)
        acc[b] += res.results[core]["outB"].astype(np.float32)
    full = np.stack(acc)
    full += b_proj[None, None, :]
    return full.astype(np.float32)
